# revision 1
# baseline (speedup 1.0000x reference)
"""Trainium2 Bass kernel for GQA MultiHeadAttention (B=1, S=2048, D=4096,
H=32 query heads, HKV=8 kv heads, DK=DV=128), tensor-parallel across heads
on 8 NeuronCores.

Sharding: core c owns query heads 4c..4c+3 and kv head c (GQA group) and
computes its 512 attention features. The transposed attention output is
AllGathered across cores in four per-q-block collectives (overlapped with
attention compute of later blocks), then each core computes a 512-row slice
of the transposed output projection. Host side: pre-transpose/cast inputs,
final concat + transpose.

Self-contained: hardcodes all shapes; inputs are the full unsharded tensors
keyed as in the problem's setup_inputs().
"""

import numpy as np
import ml_dtypes

import concourse.bacc as bacc
import concourse.mybir as mybir
from concourse.tile import TileContext
from concourse.bass_utils import run_bass_kernel_spmd

BF16 = mybir.dt.bfloat16
F32 = mybir.dt.float32

N_CORES = 8
S = 2048            # sequence length
D = 4096            # model dim
DK = 128            # head dim
NH_LOC = 4          # query heads per core
FLOC = NH_LOC * DK  # per-core attention features (512)
NDC = D // 128      # contraction chunks of 128 over D (32)
SB = 512            # q/s block width
NSB = S // SB       # 4
NST = S // 128      # 16 seq tiles of 128

_DMA_TYPES = ("InstDMACopy", "InstDMATranspose")


def _legalize_dma_waits(nc):
    """DMA pseudo-instructions encode at most ONE sem wait (the ISA events
    slot). If Tile's sem assignment leaves more on a DMA, walrus rejects it
    ("Too many sync wait commands"). Hoist all but the last wait onto fresh
    nop instructions inserted immediately before the DMA on the same engine —
    the sequencer executes them in order, so semantics are identical."""
    ctr = 0
    for f in nc.m.functions:
        for blk in f.blocks:
            out = []
            changed = False
            for inst in blk.instructions:
                si = inst.sync_info
                if (
                    si is not None
                    and len(si.on_wait) > 1
                    and type(inst).__name__ in _DMA_TYPES
                ):
                    waits = list(si.on_wait)
                    for w in waits[:-1]:
                        nop = mybir.InstNoOp(
                            name=f"I-dmawaitfix-{ctr}", ins=[], outs=[]
                        )
                        ctr += 1
                        nop.engine = inst.engine
                        nop.sync_info = mybir.SyncInfo(on_wait=[w], on_update=[])
                        out.append(nop)
                    inst.sync_info = mybir.SyncInfo(
                        on_wait=[waits[-1]], on_update=list(si.on_update)
                    )
                    changed = True
                out.append(inst)
            if changed:
                blk.instructions = out
    return ctr


def _build(stage=4, nrep=1):
    # stage: 1=projections only, 2=+attention (no collective), 4=full
    nc = bacc.Bacc("TRN2", target_bir_lowering=False, num_devices=N_CORES,
                   dynamic_dma_scratch_size=2048)

    # ---- I/O ----
    qT = nc.dram_tensor("qT", [D, S], BF16, kind="ExternalInput")
    kT = nc.dram_tensor("kT", [D, S], BF16, kind="ExternalInput")
    vT = nc.dram_tensor("vT", [D, S], BF16, kind="ExternalInput")
    wqT = nc.dram_tensor("wqT", [D, FLOC], BF16, kind="ExternalInput")
    wkT = nc.dram_tensor("wkT", [D, DK], BF16, kind="ExternalInput")
    wvT = nc.dram_tensor("wvT", [D, DK], BF16, kind="ExternalInput")
    wdT = nc.dram_tensor("wdT", [D, FLOC], BF16, kind="ExternalInput")
    masks = nc.dram_tensor("masks", [128, 4 * SB], BF16, kind="ExternalInput")
    ident = nc.dram_tensor("ident", [128, 128], BF16, kind="ExternalInput")
    outT = nc.dram_tensor("outT", [FLOC, S], F32, kind="ExternalOutput")

    with TileContext(nc) as tc:
        with (
            tc.tile_pool(name="consts", bufs=1) as consts,
            tc.tile_pool(name="bigw", bufs=1) as bigw,
            tc.tile_pool(name="persist", bufs=1) as persist,
            tc.tile_pool(name="qstream", bufs=8) as qstream,
            tc.tile_pool(name="kstream", bufs=4) as kstream,
            tc.tile_pool(name="vstream", bufs=3) as vstream,
            tc.tile_pool(name="epool", bufs=2) as epool,
            tc.tile_pool(name="small", bufs=2) as small,
            tc.tile_pool(name="attnout", bufs=2) as attnout,
            tc.tile_pool(name="atin", bufs=6) as atin,
            tc.tile_pool(name="osb", bufs=2) as osb,
            tc.tile_pool(name="ps", bufs=4, space="PSUM") as ps,
            tc.tile_pool(name="pspv", bufs=2, space="PSUM") as pspv,
            tc.tile_pool(name="psden", bufs=2, space="PSUM") as psden,
            tc.tile_pool(name="dram", bufs=1, space="DRAM") as dram,
        ):
            def one_rep(rep):
                # ---- constants into SBUF ----
                wk_sb = consts.tile([128, NDC, DK], BF16, name="wk_sb")
                nc.sync.dma_start(wk_sb[:], wkT.rearrange("(c p) f -> p c f", p=128))
                wv_sb = consts.tile([128, NDC, DK], BF16, name="wv_sb")
                nc.sync.dma_start(wv_sb[:], wvT.rearrange("(c p) f -> p c f", p=128))
                ident_sb = consts.tile([128, 128], BF16, name="ident_sb")
                nc.sync.dma_start(ident_sb[:], ident[:])
                ones_sb = consts.tile([128, 128], BF16, name="ones_sb")
                nc.vector.memset(ones_sb[:], 1.0)
                # wq early so Q-proj matmuls can fill K/V DMA-wait bubbles
                wq_sb = bigw.tile([128, NDC, FLOC], BF16, name="wq_sb", tag="bigw")
                nc.sync.dma_start(wq_sb[:], wqT.rearrange("(c p) f -> p c f", p=128))
                masks_sb = consts.tile([128, 4 * SB], BF16, name="masks_sb")
                nc.sync.dma_start(masks_sb[:], masks[:])

                # persistent activations
                QT_sb = persist.tile([128, NH_LOC, S], BF16, name="QT_sb")
                KT_sb = persist.tile([128, S], BF16, name="KT_sb")
                V_sb = persist.tile([128, NST, DK], BF16, name="V_sb")

                # per-q-block DRAM bounce buffers for the collectives
                attn_loc = [
                    dram.tile([FLOC, SB], BF16, name=f"attn_loc{qb}", tag=f"al{qb}")
                    for qb in range(NSB)
                ]
                attn_gath = [
                    dram.tile([N_CORES * FLOC, SB], BF16, name=f"attn_gath{qb}",
                              tag=f"ag{qb}", addr_space="Shared")
                    for qb in range(NSB)
                ]

                # ---- K projection: KT_sb[dk, s] ----
                k_ps = [ps.tile([128, SB], F32, name=f"kps{i}", tag="ps") for i in range(NSB)]
                for dc in range(NDC):
                    kt_c = kstream.tile([128, S], BF16, name="kt_c", tag="kt")
                    nc.sync.dma_start(kt_c[:], kT[dc * 128:(dc + 1) * 128, :])
                    for sb in range(NSB):
                        nc.tensor.matmul(
                            k_ps[sb][:],
                            lhsT=wk_sb[:, dc, :],
                            rhs=kt_c[:, sb * SB:(sb + 1) * SB],
                            start=(dc == 0),
                            stop=(dc == NDC - 1),
                        )
                for sb in range(NSB):
                    nc.vector.tensor_copy(KT_sb[:, sb * SB:(sb + 1) * SB], k_ps[sb][:])

                # ---- V projection: VT psum -> sbuf, then PE-transpose to V_sb[s, dv] ----
                VT_sb = persist.tile([128, S], BF16, name="VT_sb")
                v_ps = [ps.tile([128, SB], F32, name=f"vps{i}", tag="ps") for i in range(NSB)]
                for dc in range(NDC):
                    vt_c = vstream.tile([128, S], BF16, name="vt_c", tag="vt")
                    nc.sync.dma_start(vt_c[:], vT[dc * 128:(dc + 1) * 128, :])
                    for sb in range(NSB):
                        nc.tensor.matmul(
                            v_ps[sb][:],
                            lhsT=wv_sb[:, dc, :],
                            rhs=vt_c[:, sb * SB:(sb + 1) * SB],
                            start=(dc == 0),
                            stop=(dc == NDC - 1),
                        )
                for sb in range(NSB):
                    nc.vector.tensor_copy(VT_sb[:, sb * SB:(sb + 1) * SB], v_ps[sb][:])
                for st in range(NST):
                    tp = ps.tile([128, 128], BF16, name="tp", tag="ps")
                    nc.tensor.transpose(tp[:], VT_sb[:, st * 128:(st + 1) * 128], ident_sb[:])
                    nc.vector.tensor_copy(V_sb[:, st, :], tp[:])

                # ---- Q projection: single pass over qT; accumulate Dc-blocks of 8
                # in PSUM, partial-sum into fp32 QTacc, final block writes bf16 QT_sb.
                # Q psums borrow the attention pv/den pools (idle this early).
                QTacc = persist.tile([128, NH_LOC, S], F32, name="QTacc")
                NBLK = 4
                BLK = NDC // NBLK  # 8 chunks per block
                for dcb in range(NBLK):
                    q_chunks = []
                    for i in range(BLK):
                        dc = dcb * BLK + i
                        qt_c = qstream.tile([128, S], BF16, name="qt_c", tag="qt")
                        nc.sync.dma_start(qt_c[:], qT[dc * 128:(dc + 1) * 128, :])
                        q_chunks.append(qt_c)
                    for f in range(NH_LOC):
                        if f % 2 == 0:
                            q_ps = [
                                pspv.tile([128, SB], F32, name=f"qps{s2}", tag="pv")
                                if s2 < 2 else
                                psden.tile([128, SB], F32, name=f"qps{s2}", tag="den")
                                for s2 in range(NSB)
                            ]
                        else:
                            q_ps = [ps.tile([128, SB], F32, name=f"qps{s2}", tag="ps")
                                    for s2 in range(NSB)]
                        for i in range(BLK):
                            dc = dcb * BLK + i
                            for s2 in range(NSB):
                                nc.tensor.matmul(
                                    q_ps[s2][:],
                                    lhsT=wq_sb[:, dc, f * 128:(f + 1) * 128],
                                    rhs=q_chunks[i][:, s2 * SB:(s2 + 1) * SB],
                                    start=(i == 0),
                                    stop=(i == BLK - 1),
                                )
                        for s2 in range(NSB):
                            dst_acc = QTacc[:, f, s2 * SB:(s2 + 1) * SB]
                            if dcb == 0:
                                nc.vector.tensor_copy(dst_acc, q_ps[s2][:])
                            elif dcb < NBLK - 1:
                                nc.vector.tensor_tensor(
                                    dst_acc, dst_acc, q_ps[s2][:], mybir.AluOpType.add
                                )
                            else:
                                nc.vector.tensor_tensor(
                                    QT_sb[:, f, s2 * SB:(s2 + 1) * SB],
                                    dst_acc, q_ps[s2][:], mybir.AluOpType.add,
                                )

                if stage == 1:
                    for hh in range(NH_LOC):
                        for sb in range(NSB):
                            o_sb = osb.tile([128, SB], F32, name="o_sb", tag="osb")
                            nc.vector.tensor_copy(o_sb[:], QT_sb[:, hh, sb * SB:(sb + 1) * SB])
                            nc.sync.dma_start(
                                outT[hh * 128:(hh + 1) * 128, sb * SB:(sb + 1) * SB], o_sb[:]
                            )

                # wd loads during attention (shares the wq slot; Tile orders it
                # after the last wq read)
                if stage >= 2:
                    wd_sb = bigw.tile([128, NDC, FLOC], BF16, name="wd_sb", tag="bigw")
                    nc.sync.dma_start(wd_sb[:], wdT.rearrange("(c p) f -> p c f", p=128))

                # ---- attention, q-block outer so each block's AllGather can
                # ---- overlap later blocks' compute
                if stage >= 2:
                    for qb in range(NSB):
                        nkt = 4 * qb + 4  # causal: k-tiles 0..4qb+3
                        for h in range(NH_LOC):
                            E = epool.tile([128, NST, SB], BF16, name="E", tag="E")
                            for kt in range(nkt):
                                st_ps = ps.tile([128, SB], F32, name="st_ps", tag="ps")
                                nc.tensor.matmul(
                                    st_ps[:],
                                    lhsT=KT_sb[:, kt * 128:(kt + 1) * 128],
                                    rhs=QT_sb[:, h, qb * SB:(qb + 1) * SB],
                                    start=True,
                                    stop=True,
                                )
                                nc.scalar.activation(
                                    E[:, kt, :], st_ps[:], mybir.ActivationFunctionType.Exp
                                )
                                d = kt - 4 * qb
                                if d >= 0:  # diagonal tile -> causal mask
                                    nc.vector.tensor_tensor(
                                        E[:, kt, :],
                                        E[:, kt, :],
                                        masks_sb[:, d * SB:(d + 1) * SB],
                                        mybir.AluOpType.mult,
                                    )
                            den_ps = psden.tile([128, SB], F32, name="den_ps", tag="den")
                            att_ps = pspv.tile([128, SB], F32, name="att_ps", tag="pv")
                            for kt in range(nkt):
                                nc.tensor.matmul(
                                    den_ps[:],
                                    lhsT=ones_sb[:, :],
                                    rhs=E[:, kt, :],
                                    start=(kt == 0),
                                    stop=(kt == nkt - 1),
                                )
                                nc.tensor.matmul(
                                    att_ps[:],
                                    lhsT=V_sb[:, kt, :],
                                    rhs=E[:, kt, :],
                                    start=(kt == 0),
                                    stop=(kt == nkt - 1),
                                )
                            # normalize: attn[dv, q] /= den[q]. den_ps was computed
                            # with a full ones-matrix lhsT, so every PSUM partition
                            # holds the denominator row -> plain elementwise multiply.
                            rec = small.tile([128, SB], F32, name="rec", tag="rec")
                            nc.vector.reciprocal(rec[:], den_ps[:])
                            attn_t = attnout.tile([128, SB], BF16, name="attn_t", tag="attn")
                            nc.vector.tensor_tensor(
                                attn_t[:], att_ps[:], rec[:], mybir.AluOpType.mult
                            )
                            nc.sync.dma_start(
                                attn_loc[qb][h * 128:(h + 1) * 128, :], attn_t[:]
                            )
                            if stage == 2:
                                o_sb = osb.tile([128, SB], F32, name="o_sb", tag="osb2")
                                nc.vector.tensor_copy(o_sb[:], attn_t[:])
                                nc.sync.dma_start(
                                    outT[h * 128:(h + 1) * 128, qb * SB:(qb + 1) * SB],
                                    o_sb[:],
                                )

                        if stage >= 4:
                            # gather this q-block's attn^T from all cores
                            nc.gpsimd.collective_compute(
                                "AllGather",
                                mybir.AluOpType.bypass,
                                replica_groups=[list(range(N_CORES))],
                                ins=[attn_loc[qb][:]],
                                outs=[attn_gath[qb][:]],
                            )

                # ---- output projection per q-block: outT[d, qb] (512x512 slice) ----
                if stage >= 4:
                    for qb in range(NSB):
                        o_ps = [
                            ps.tile([128, SB], F32, name=f"ops{d2}", tag="ps")
                            if d2 < 2 else
                            (pspv.tile([128, SB], F32, name=f"ops{d2}", tag="pv")
                             if d2 == 2 else
                             psden.tile([128, SB], F32, name=f"ops{d2}", tag="den"))
                            for d2 in range(4)
                        ]
                        for fc in range(NDC):
                            at_c = atin.tile([128, SB], BF16, name="at_c", tag="atin")
                            nc.sync.dma_start(
                                at_c[:], attn_gath[qb][fc * 128:(fc + 1) * 128, :]
                            )
                            for dsub in range(4):
                                nc.tensor.matmul(
                                    o_ps[dsub][:],
                                    lhsT=wd_sb[:, fc, dsub * 128:(dsub + 1) * 128],
                                    rhs=at_c[:],
                                    start=(fc == 0),
                                    stop=(fc == NDC - 1),
                                )
                        for dsub in range(4):
                            o_sb = osb.tile([128, SB], F32, name="o_sb", tag="osb")
                            nc.vector.tensor_copy(o_sb[:], o_ps[dsub][:])
                            nc.sync.dma_start(
                                outT[dsub * 128:(dsub + 1) * 128,
                                     qb * SB:(qb + 1) * SB],
                                o_sb[:],
                            )

            for rep in range(nrep):
                one_rep(rep)

    nc.compile()
    _legalize_dma_waits(nc)
    nc.codegen_inst_isa_subclasses()
    return nc


_NC_CACHE = None


def _get_nc():
    global _NC_CACHE
    if _NC_CACHE is None:
        _NC_CACHE = _build()
    return _NC_CACHE


def _make_in_maps(q, k, v, Wq, Wk, Wv, Wd):
    bf = ml_dtypes.bfloat16
    scale = np.float32(DK) ** -0.5
    qT = np.ascontiguousarray(q.reshape(S, D).T).astype(bf)
    kT = np.ascontiguousarray(k.reshape(S, D).T).astype(bf)
    vT = np.ascontiguousarray(v.reshape(S, D).T).astype(bf)

    kp = np.arange(128, dtype=np.int32)[:, None]
    qf = np.arange(SB, dtype=np.int32)[None, :]
    masks = np.concatenate(
        [(qf >= kp + 128 * d).astype(np.float32) for d in range(4)], axis=1
    ).astype(bf)
    ident = np.eye(128, dtype=np.float32).astype(bf)

    in_maps = []
    for c in range(N_CORES):
        fs = slice(FLOC * c, FLOC * (c + 1))
        ks = slice(DK * c, DK * (c + 1))
        in_maps.append({
            "qT": qT,
            "kT": kT,
            "vT": vT,
            "wqT": np.ascontiguousarray((Wq[fs, :] * scale).T).astype(bf),
            "wkT": np.ascontiguousarray(Wk[ks, :].T).astype(bf),
            "wvT": np.ascontiguousarray(Wv[ks, :].T).astype(bf),
            "wdT": np.ascontiguousarray(Wd[fs, :].T).astype(bf),
            "masks": masks,
            "ident": ident,
        })
    return in_maps


def _assemble(results):
    outT_full = np.concatenate([r["outT"] for r in results], axis=0)  # [4096, 2048]
    return np.ascontiguousarray(outT_full.T).reshape(1, S, D).astype(np.float32)


def kernel(q, k, v, Wq, Wk, Wv, Wd, _trace=False, **_ignored):
    nc = _get_nc()
    in_maps = _make_in_maps(
        np.asarray(q, np.float32), np.asarray(k, np.float32),
        np.asarray(v, np.float32), np.asarray(Wq, np.float32),
        np.asarray(Wk, np.float32), np.asarray(Wv, np.float32),
        np.asarray(Wd, np.float32),
    )
    res = run_bass_kernel_spmd(
        nc, in_maps, core_ids=list(range(N_CORES)), trace=_trace
    )
    out = _assemble(res.results)
    if _trace:
        return out, res
    return out



# revision 28
# speedup vs baseline: 38913.3090x; 38913.3090x over previous
"""Trainium2 Bass kernel for GQA MultiHeadAttention (B=1, S=2048, D=4096,
H=32 query heads, HKV=8 kv heads, DK=DV=128) on 8 NeuronCores.

Sharding: core c owns query heads 4c..4c+3 and kv head c for the projections
and attention (tensor-parallel over heads); the output projection is
sequence-sharded: a per-head AllToAll redistributes the attention output so
core c holds all 4096 attention features for its 256 sequence columns, then
each core computes out[:, own 256 cols] against the full (permuted) Wd.

Phase layout per core:
  P1 interleaved projections: per 8-chunk block of the D contraction,
     K, V, Qh0..3 round-robin over two 2-bank PSUM pools, partial sums
     accumulated in SBUF; kT/vT/qT chunks stream interleaved so the PE
     never waits on any single tensor's DMA.
  P2 attention, head-outer: scores in double-buffered groups of 2 k-tiles
     -> batched exp; causal mask on DVE; PV accumulated in PSUM across the
     row; softmax denominator via ones-matmul (heads 0,2) or DVE
     accumulation (heads 1,3) to balance PE/DVE load. After each head:
     AllToAll of its attention output.
  P3 output projection, one pass per head-group, od-quarters so wd
     sub-chunks stay resident across both q-halves; accumulates into an
     SBUF fp32 buffer reusing the Q-accumulator slot; full Wd streamed
     through the same pool as the qT stream.

Self-contained: hardcodes all shapes; inputs are the full unsharded tensors
keyed as in the problem's setup_inputs().
"""

import numpy as np
import ml_dtypes

import concourse.bacc as bacc
import concourse.mybir as mybir
from concourse.tile import TileContext
from concourse.bass_utils import run_bass_kernel_spmd

BF16 = mybir.dt.bfloat16
F32 = mybir.dt.float32

N_CORES = 8
S = 2048            # sequence length
D = 4096            # model dim
DK = 128            # head dim
NH_LOC = 4          # query heads per core
FLOC = NH_LOC * DK  # per-core attention features (512)
NDC = D // 128      # contraction chunks of 128 over D (32)
SB = 512            # q/s block width
NSB = S // SB       # 4
NST = S // 128      # 16 seq tiles of 128
SLOC = S // N_CORES  # per-core output seq columns (256)
NBLK = 4            # projection blocks
BLK = NDC // NBLK   # 8 dc per block
OQ = 1024           # output-projection od quarter width

_DMA_TYPES = ("InstDMACopy", "InstDMATranspose")


def _legalize_dma_waits(nc):
    """DMA pseudo-instructions encode at most ONE sem wait (the ISA events
    slot). If Tile's sem assignment leaves more on a DMA, walrus rejects it
    ("Too many sync wait commands"). Hoist all but the last wait onto fresh
    nop instructions inserted immediately before the DMA on the same engine —
    the sequencer executes them in order, so semantics are identical."""
    ctr = 0
    for f in nc.m.functions:
        for blk in f.blocks:
            out = []
            changed = False
            for inst in blk.instructions:
                si = inst.sync_info
                if (
                    si is not None
                    and len(si.on_wait) > 1
                    and type(inst).__name__ in _DMA_TYPES
                ):
                    waits = list(si.on_wait)
                    for w in waits[:-1]:
                        nop = mybir.InstNoOp(
                            name=f"I-dmawaitfix-{ctr}", ins=[], outs=[]
                        )
                        ctr += 1
                        nop.engine = inst.engine
                        nop.sync_info = mybir.SyncInfo(on_wait=[w], on_update=[])
                        out.append(nop)
                    inst.sync_info = mybir.SyncInfo(
                        on_wait=[waits[-1]], on_update=list(si.on_update)
                    )
                    changed = True
                out.append(inst)
            if changed:
                blk.instructions = out
    return ctr


def _build(nrep=1):
    nc = bacc.Bacc("TRN2", target_bir_lowering=False, num_devices=N_CORES,
                   dynamic_dma_scratch_size=2048)

    # ---- I/O (host pre-layouts everything partition-major) ----
    qr = nc.dram_tensor("qr", [128, NDC, S], BF16, kind="ExternalInput")
    kr = nc.dram_tensor("kr", [128, NDC, S], BF16, kind="ExternalInput")
    vr = nc.dram_tensor("vr", [128, NDC, S], BF16, kind="ExternalInput")
    wq = nc.dram_tensor("wq", [128, NDC, FLOC], BF16, kind="ExternalInput")
    wk = nc.dram_tensor("wk", [128, NDC, DK], BF16, kind="ExternalInput")
    wv = nc.dram_tensor("wv", [128, NDC, DK], BF16, kind="ExternalInput")
    wd = nc.dram_tensor("wd", [128, NDC, D], BF16, kind="ExternalInput")
    masks = nc.dram_tensor("masks", [128, 4, SB], BF16, kind="ExternalInput")
    ident = nc.dram_tensor("ident", [128, 128], BF16, kind="ExternalInput")
    outS = nc.dram_tensor("outS", [SLOC, D], F32, kind="ExternalOutput")

    with TileContext(nc) as tc:
        with (
            tc.tile_pool(name="consts", bufs=1) as consts,
            tc.tile_pool(name="wqp", bufs=2) as wqp,
            tc.tile_pool(name="wkvp", bufs=2) as wkvp,
            tc.tile_pool(name="streamA", bufs=12) as streamA,
            tc.tile_pool(name="ktp", bufs=4) as ktp,
            tc.tile_pool(name="vtp", bufs=4) as vtp,
            tc.tile_pool(name="kaccp", bufs=1) as kaccp,
            tc.tile_pool(name="bigacc", bufs=1) as bigacc,
            tc.tile_pool(name="finals", bufs=1) as finals,
            tc.tile_pool(name="epool", bufs=3) as epool,
            tc.tile_pool(name="eaccp", bufs=2) as eaccp,
            tc.tile_pool(name="recp", bufs=2) as recp,
            tc.tile_pool(name="atout", bufs=2) as atout,
            tc.tile_pool(name="featp", bufs=2) as featp,
            tc.tile_pool(name="psA", bufs=2, space="PSUM") as psA,
            tc.tile_pool(name="psB", bufs=2, space="PSUM") as psB,
            tc.tile_pool(name="dram", bufs=1, space="DRAM") as dram,
        ):
            def one_rep(rep):
                ones_sb = consts.tile([128, 128], BF16, name="ones_sb")
                nc.vector.memset(ones_sb[:], 1.0)
                # PE warmup: dummy matmuls ramp the PE p-state while the
                # first stream DMAs are in flight
                warm = psB.tile([128, 2, SB], F32, name="warm", tag="psB")
                for w in range(24):
                    nc.tensor.matmul(warm[:, w % 2, 0:128], lhsT=ones_sb[:],
                                     rhs=ones_sb[:], start=(w < 2),
                                     stop=(w >= 22))

                # persistent activations
                QT_sb = finals.tile([128, NH_LOC, S], BF16, name="QT_sb")
                KT_sb = finals.tile([128, S], BF16, name="KT_sb")
                VT_sb = finals.tile([128, S], BF16, name="VT_sb")
                V_sb = finals.tile([128, NST, DK], BF16, name="V_sb")
                KTacc = kaccp.tile([128, S], F32, name="KTacc")
                QTacc = bigacc.tile([128, 4, S], F32, name="QTacc", tag="big")

                # a2a bounce buffers (one per local head)
                ain = [dram.tile([N_CORES, 128, SLOC], BF16,
                                 name=f"ain{h}", tag=f"ain{h}")
                       for h in range(NH_LOC)]
                aout = [dram.tile([N_CORES, 128, SLOC], BF16,
                                  name=f"aout{h}", tag=f"aout{h}")
                        for h in range(NH_LOC)]

                def drain(dst_flat, na, srcs, mode):
                    # dst_flat: AP [128, na*SB*len(srcs)]; srcs: psum tiles
                    # [128, na, SB]; mode: "copy" | "add" | None->into dst
                    for t, src in enumerate(srcs):
                        dst = dst_flat[:, t * na * SB:(t + 1) * na * SB] \
                            .rearrange("p (a b) -> p a b", a=na)
                        if mode == "copy":
                            nc.vector.tensor_copy(dst, src[:])
                        else:
                            nc.vector.tensor_tensor(dst, dst, src[:],
                                                    mybir.AluOpType.add)

                # ---- P1: interleaved projections ----
                for blk in range(NBLK):
                    dc0 = blk * BLK
                    # K weights + K stream first (first consumers), then V, Q
                    wkc = wkvp.tile([128, BLK, DK], BF16, name="wkc", tag="wkc")
                    nc.sync.dma_start(wkc[:], wk[:, dc0:dc0 + BLK, :])
                    kt = []
                    vt = []
                    qt = []
                    for i in range(BLK):
                        t = ktp.tile([128, S], BF16, name="kt_c", tag="kt")
                        nc.sync.dma_start(t[:], kr[:, dc0 + i, :])
                        kt.append(t)
                    wvc = wkvp.tile([128, BLK, DK], BF16, name="wvc", tag="wvc")
                    nc.sync.dma_start(wvc[:], wv[:, dc0:dc0 + BLK, :])
                    wqc = wqp.tile([128, BLK, FLOC], BF16, name="wqc", tag="wqc")
                    nc.sync.dma_start(wqc[:], wq[:, dc0:dc0 + BLK, :])
                    for i in range(BLK):
                        t = vtp.tile([128, S], BF16, name="vt_c", tag="vt")
                        nc.sync.dma_start(t[:], vr[:, dc0 + i, :])
                        vt.append(t)
                    for i in range(BLK):
                        t = streamA.tile([128, S], BF16, name="qt_c", tag="sa")
                        nc.sync.dma_start(t[:], qr[:, dc0 + i, :])
                        qt.append(t)

                    # K -> psA pair
                    kps = [psA.tile([128, 2, SB], F32, name=f"kps{t}", tag="psA")
                           for t in range(2)]
                    for i in range(BLK):
                        for sb in range(NSB):
                            nc.tensor.matmul(
                                kps[sb // 2][:, sb % 2, :],
                                lhsT=wkc[:, i, :],
                                rhs=kt[i][:, sb * SB:(sb + 1) * SB],
                                start=(i == 0), stop=(i == BLK - 1),
                            )
                    if blk == 0:
                        drain(KTacc[:], 2, kps, "copy")
                    elif blk < NBLK - 1:
                        drain(KTacc[:], 2, kps, "add")
                    else:
                        for t in range(2):
                            dst = KT_sb[:, t * 2 * SB:(t + 1) * 2 * SB] \
                                .rearrange("p (a b) -> p a b", a=2)
                            acc = KTacc[:, t * 2 * SB:(t + 1) * 2 * SB] \
                                .rearrange("p (a b) -> p a b", a=2)
                            nc.vector.tensor_tensor(dst, acc, kps[t][:],
                                                    mybir.AluOpType.add)

                    # V -> psB pair, bf16 accumulate directly in VT_sb
                    vps = [psB.tile([128, 2, SB], F32, name=f"vps{t}", tag="psB")
                           for t in range(2)]
                    for i in range(BLK):
                        for sb in range(NSB):
                            nc.tensor.matmul(
                                vps[sb // 2][:, sb % 2, :],
                                lhsT=wvc[:, i, :],
                                rhs=vt[i][:, sb * SB:(sb + 1) * SB],
                                start=(i == 0), stop=(i == BLK - 1),
                            )
                    drain(VT_sb[:], 2, vps, "copy" if blk == 0 else "add")

                    # Q heads: h0,h2 -> psA pair; h1,h3 -> psB pair
                    for f in range(NH_LOC):
                        pool = psA if f % 2 == 0 else psB
                        tag = "psA" if f % 2 == 0 else "psB"
                        qps = [pool.tile([128, 2, SB], F32, name=f"qps{t}",
                                         tag=tag) for t in range(2)]
                        for i in range(BLK):
                            for sb in range(NSB):
                                nc.tensor.matmul(
                                    qps[sb // 2][:, sb % 2, :],
                                    lhsT=wqc[:, i, f * 128:(f + 1) * 128],
                                    rhs=qt[i][:, sb * SB:(sb + 1) * SB],
                                    start=(i == 0), stop=(i == BLK - 1),
                                )
                        if blk == 0:
                            drain(QTacc[:, f, :], 2, qps, "copy")
                        elif blk < NBLK - 1:
                            drain(QTacc[:, f, :], 2, qps, "add")
                        else:
                            for t in range(2):
                                dst = QT_sb[:, f, t * 2 * SB:(t + 1) * 2 * SB] \
                                    .rearrange("p (a b) -> p a b", a=2)
                                acc = QTacc[:, f, t * 2 * SB:(t + 1) * 2 * SB] \
                                    .rearrange("p (a b) -> p a b", a=2)
                                nc.vector.tensor_tensor(dst, acc, qps[t][:],
                                                        mybir.AluOpType.add)

                # V transposes: VT_sb [dv, s] -> V_sb [s, kt, dv]
                ident_sb = consts.tile([128, 128], BF16, name="ident_sb")
                nc.sync.dma_start(ident_sb[:], ident[:])
                masks_sb = consts.tile([128, 4, SB], BF16, name="masks_sb")
                nc.sync.dma_start(masks_sb[:], masks[:])
                for t in range(2):
                    tp = psB.tile([128, 2, SB], BF16, name="tp", tag="psB")
                    for i in range(8):
                        st = t * 8 + i
                        nc.tensor.transpose(
                            tp[:, i // 4, (i % 4) * 128:(i % 4 + 1) * 128],
                            VT_sb[:, st * 128:(st + 1) * 128], ident_sb[:])
                    nc.vector.tensor_copy(
                        V_sb[:, t * 8:(t + 1) * 8, :].rearrange(
                            "p (a b) c -> p a (b c)", a=2),
                        tp[:])

                # prefetch pass-0 wd pair-chunks while attention runs (SP
                # queue is past all P1 stream DMAs at this point)
                wdpre = {}
                for oqp in range(2):
                    for j in range(N_CORES):
                        if len(wdpre) >= 12:
                            break
                        t = streamA.tile([128, 2, OQ], BF16, name="wdq",
                                         tag="sa")
                        nc.sync.dma_start(
                            t[:], wd[:, 0 * 8 + j,
                                     2 * oqp * OQ:(2 * oqp + 2) * OQ]
                            .rearrange("p (a b) -> p a b", a=2))
                        wdpre[(0, oqp, j)] = t

                # ---- P2: attention, head-PAIR interleaved so two
                # independent dependency chains fill each other's bubbles
                for hp in range(NH_LOC // 2):
                    heads = (2 * hp, 2 * hp + 1)  # (PE-den head, DVE-den head)
                    for qb in range(NSB):
                        nkt = 4 * qb + 4
                        ngrp = nkt // 2
                        pvden = {}
                        for h in heads:
                            pvden[h] = psB.tile([128, 2, SB], F32,
                                                name=f"pvden{h}", tag="psB")
                        eacc = {h: eaccp.tile([128, 2, SB], BF16,
                                               name=f"eacc{h}", tag=f"eacc{h % 2}")
                                for h in heads}
                        order = list(range(ngrp))
                        if ngrp > 2:  # diagonal (masked) groups first
                            order = [ngrp - 2, ngrp - 1] + list(range(ngrp - 2))
                        for pos, g in enumerate(order):
                            first, last = pos == 0, pos == ngrp - 1
                            # second diagonal group: cols < SLOC are fully
                            # masked; skip them in exp/mask/PV/eacc entirely
                            rq = SLOC if (g == ngrp - 1 and ngrp > 2) else 0
                            E = {}
                            for h in heads:
                                sc = psA.tile([128, 2, SB], F32, name="sc",
                                              tag="psA")
                                for i in range(2):
                                    kt_i = 2 * g + i
                                    nc.tensor.matmul(
                                        sc[:, i, :],
                                        lhsT=KT_sb[:, kt_i * 128:
                                                   (kt_i + 1) * 128],
                                        rhs=QT_sb[:, h,
                                                  qb * SB:(qb + 1) * SB],
                                        start=True, stop=True,
                                    )
                                E[h] = epool.tile([128, 2, SB], BF16,
                                                  name="E", tag="E")
                                nc.scalar.activation(
                                    E[h][:, :, rq:SB], sc[:, :, rq:SB],
                                    mybir.ActivationFunctionType.Exp)
                                if g >= ngrp - 2:  # diagonal pair -> mask
                                    u = g - (ngrp - 2)
                                    nc.vector.tensor_tensor(
                                        E[h][:, :, rq:SB], E[h][:, :, rq:SB],
                                        masks_sb[:, 2 * u:2 * u + 2, rq:SB],
                                        mybir.AluOpType.mult)
                            for h in heads:
                                for i in range(2):
                                    nc.tensor.matmul(
                                        pvden[h][:, 0, rq:SB],
                                        lhsT=V_sb[:, 2 * g + i, :],
                                        rhs=E[h][:, i, rq:SB],
                                        start=(first and i == 0),
                                        stop=(last and i == 1),
                                    )
                            # denominators via DVE accumulation (both heads)
                            ha, hb = heads
                            for h in heads:
                                if first:
                                    nc.vector.tensor_copy(eacc[h][:], E[h][:])
                                else:
                                    nc.vector.tensor_tensor(
                                        eacc[h][:, :, rq:SB],
                                        eacc[h][:, :, rq:SB],
                                        E[h][:, :, rq:SB],
                                        mybir.AluOpType.add)
                        for h in heads:
                            nc.tensor.matmul(pvden[h][:, 1, :],
                                             lhsT=ones_sb[:],
                                             rhs=eacc[h][:, 0, :],
                                             start=True, stop=False)
                            nc.tensor.matmul(pvden[h][:, 1, :],
                                             lhsT=ones_sb[:],
                                             rhs=eacc[h][:, 1, :],
                                             start=False, stop=True)
                        for h in heads:
                            rec = recp.tile([128, SB], F32, name="rec",
                                            tag="rec")
                            nc.vector.reciprocal(rec[:], pvden[h][:, 1, :])
                            attn_t = atout.tile([128, SB], BF16,
                                                name="attn_t", tag="attn")
                            nc.vector.tensor_tensor(
                                attn_t[:], pvden[h][:, 0, :], rec[:],
                                mybir.AluOpType.mult)
                            nc.sync.dma_start(ain[h][2 * qb],
                                              attn_t[:, 0:SLOC])
                            nc.sync.dma_start(ain[h][2 * qb + 1],
                                              attn_t[:, SLOC:SB])
                    for h in heads:
                        nc.gpsimd.collective_compute(
                            "AllToAll",
                            mybir.AluOpType.bypass,
                            replica_groups=[list(range(N_CORES))],
                            ins=[ain[h][:]],
                            outs=[aout[h][:]],
                        )

                # ---- P3: output projection, one pass per head-group ----
                out_acc = bigacc.tile([128, 2, D], F32, name="out_acc",
                                      tag="big")
                for h in range(NH_LOC):
                    feats = featp.tile([128, N_CORES, SLOC], BF16,
                                       name="feats", tag="feats")
                    for j in range(N_CORES):
                        nc.sync.dma_start(feats[:, j, :], aout[h][j])
                    for oq in range(4):
                        oqp = oq // 2
                        if oq % 2 == 0:
                            wdqp = []
                            for j in range(N_CORES):
                                if (h, oqp, j) in wdpre:
                                    wdqp.append(wdpre[(h, oqp, j)])
                                    continue
                                t = streamA.tile([128, 2, OQ], BF16,
                                                 name="wdq", tag="sa")
                                nc.sync.dma_start(
                                    t[:], wd[:, h * 8 + j,
                                             2 * oqp * OQ:(2 * oqp + 2) * OQ]
                                    .rearrange("p (a b) -> p a b", a=2))
                                wdqp.append(t)
                        wdq = [wt[:, oq % 2, :] for wt in wdqp]
                        pool = psA if oq % 2 == 0 else psB
                        tag = "psA" if oq % 2 == 0 else "psB"
                        pq = [pool.tile([128, 2, SB], F32, name=f"ops{t}",
                                        tag=tag) for t in range(2)]
                        for qh in range(2):
                            for j in range(N_CORES):
                                for t in range(2):
                                    nc.tensor.matmul(
                                        pq[qh][:, t, :],
                                        lhsT=feats[:, j,
                                                   qh * 128:(qh + 1) * 128],
                                        rhs=wdq[j][:, t * SB:(t + 1) * SB],
                                        start=(j == 0),
                                        stop=(j == N_CORES - 1),
                                    )
                        for qh in range(2):
                            dst = out_acc[:, qh, oq * OQ:(oq + 1) * OQ] \
                                .rearrange("p (b c) -> p b c", c=SB)
                            if h == 0:
                                nc.vector.tensor_copy(dst, pq[qh][:])
                            else:
                                nc.vector.tensor_tensor(dst, dst, pq[qh][:],
                                                        mybir.AluOpType.add)
                            if h == NH_LOC - 1:
                                nc.sync.dma_start(
                                    outS[qh * 128:(qh + 1) * 128,
                                         oq * OQ:(oq + 1) * OQ],
                                    out_acc[:, qh, oq * OQ:(oq + 1) * OQ])

            for rep in range(nrep):
                one_rep(rep)

    nc.compile()
    _legalize_dma_waits(nc)
    nc.codegen_inst_isa_subclasses()
    return nc


_NC_CACHE = None


def _get_nc():
    global _NC_CACHE
    if _NC_CACHE is None:
        _NC_CACHE = _build()
    return _NC_CACHE


def _pm(a, nchunk, width):
    """[nchunk*128, width] -> [128, nchunk, width] partition-major bf16."""
    bf = ml_dtypes.bfloat16
    return np.ascontiguousarray(
        a.reshape(nchunk, 128, width).transpose(1, 0, 2)).astype(bf)


def _make_in_maps(q, k, v, Wq, Wk, Wv, Wd):
    bf = ml_dtypes.bfloat16
    scale = np.float32(DK) ** -0.5
    qT = np.ascontiguousarray(q.reshape(S, D).T)   # [D, S]
    kT = np.ascontiguousarray(k.reshape(S, D).T)
    vT = np.ascontiguousarray(v.reshape(S, D).T)
    qr = _pm(qT, NDC, S)
    kr = _pm(kT, NDC, S)
    vr = _pm(vT, NDC, S)

    # permuted Wd: row block (h, j) = features of global head 4j+h
    wdT = np.ascontiguousarray(Wd.T)               # [feats, od]
    blocks = []
    for h in range(NH_LOC):
        for j in range(N_CORES):
            g = 4 * j + h
            blocks.append(wdT[g * 128:(g + 1) * 128, :])
    wd_r = _pm(np.concatenate(blocks, axis=0), NDC, D)

    kp = np.arange(128, dtype=np.int32)[:, None]
    qf = np.arange(SB, dtype=np.int32)[None, :]
    masks = np.stack(
        [(qf >= kp + 128 * d).astype(np.float32) for d in range(4)], axis=1
    ).astype(bf)  # [128, 4, SB]
    ident = np.eye(128, dtype=np.float32).astype(bf)

    in_maps = []
    for c in range(N_CORES):
        fs = slice(FLOC * c, FLOC * (c + 1))
        ks = slice(DK * c, DK * (c + 1))
        in_maps.append({
            "qr": qr,
            "kr": kr,
            "vr": vr,
            "wq": _pm(np.ascontiguousarray((Wq[fs, :] * scale).T), NDC, FLOC),
            "wk": _pm(np.ascontiguousarray(Wk[ks, :].T), NDC, DK),
            "wv": _pm(np.ascontiguousarray(Wv[ks, :].T), NDC, DK),
            "wd": wd_r,
            "masks": masks,
            "ident": ident,
        })
    return in_maps


def _assemble(results):
    return np.concatenate(
        [r["outS"] for r in results], axis=0).reshape(1, S, D)


def kernel(q, k, v, Wq, Wk, Wv, Wd, _trace=False, **_ignored):
    nc = _get_nc()
    in_maps = _make_in_maps(
        np.asarray(q, np.float32), np.asarray(k, np.float32),
        np.asarray(v, np.float32), np.asarray(Wq, np.float32),
        np.asarray(Wk, np.float32), np.asarray(Wv, np.float32),
        np.asarray(Wd, np.float32),
    )
    res = run_bass_kernel_spmd(
        nc, in_maps, core_ids=list(range(N_CORES)), trace=_trace
    )
    out = _assemble(res.results)
    if _trace:
        return out, res
    return out


# revision 37
# speedup vs baseline: 39513.1154x; 1.0154x over previous
"""Trainium2 Bass kernel for GQA MultiHeadAttention (B=1, S=2048, D=4096,
H=32 query heads, HKV=8 kv heads, DK=DV=128) on 8 NeuronCores.

Sharding: core c owns query heads 4c..4c+3 and kv head c for the projections
and attention (tensor-parallel over heads); the output projection is
sequence-sharded: a per-head AllToAll redistributes the attention output so
core c holds all 4096 attention features for its 256 sequence columns, then
each core computes out[:, own 256 cols] against the full (permuted) Wd.

Phase layout per core:
  P1 interleaved projections: per 8-chunk block of the D contraction,
     K, V, Qh0..3 round-robin over two 2-bank PSUM pools, partial sums
     accumulated in SBUF; kT/vT/qT chunks stream interleaved so the PE
     never waits on any single tensor's DMA.
  P2 attention, head-outer: scores in double-buffered groups of 2 k-tiles
     -> batched exp; causal mask on DVE; PV accumulated in PSUM across the
     row; softmax denominator via ones-matmul (heads 0,2) or DVE
     accumulation (heads 1,3) to balance PE/DVE load. After each head:
     AllToAll of its attention output.
  P3 output projection, one pass per head-group, od-quarters so wd
     sub-chunks stay resident across both q-halves; accumulates into an
     SBUF fp32 buffer reusing the Q-accumulator slot; full Wd streamed
     through the same pool as the qT stream.

Self-contained: hardcodes all shapes; inputs are the full unsharded tensors
keyed as in the problem's setup_inputs().
"""

import numpy as np
import ml_dtypes

import concourse.bacc as bacc
import concourse.mybir as mybir
from concourse.tile import TileContext
from concourse.bass_utils import run_bass_kernel_spmd

BF16 = mybir.dt.bfloat16
F32 = mybir.dt.float32

N_CORES = 8
S = 2048            # sequence length
D = 4096            # model dim
DK = 128            # head dim
NH_LOC = 4          # query heads per core
FLOC = NH_LOC * DK  # per-core attention features (512)
NDC = D // 128      # contraction chunks of 128 over D (32)
SB = 512            # q/s block width
NSB = S // SB       # 4
NST = S // 128      # 16 seq tiles of 128
SLOC = S // N_CORES  # per-core output seq columns (256)
NBLK = 4            # projection blocks
BLK = NDC // NBLK   # 8 dc per block
OQ = 1024           # output-projection od quarter width

_DMA_TYPES = ("InstDMACopy", "InstDMATranspose")


def _legalize_dma_waits(nc):
    """DMA pseudo-instructions encode at most ONE sem wait (the ISA events
    slot). If Tile's sem assignment leaves more on a DMA, walrus rejects it
    ("Too many sync wait commands"). Hoist all but the last wait onto fresh
    nop instructions inserted immediately before the DMA on the same engine —
    the sequencer executes them in order, so semantics are identical."""
    ctr = 0
    for f in nc.m.functions:
        for blk in f.blocks:
            out = []
            changed = False
            for inst in blk.instructions:
                si = inst.sync_info
                if (
                    si is not None
                    and len(si.on_wait) > 1
                    and type(inst).__name__ in _DMA_TYPES
                ):
                    waits = list(si.on_wait)
                    for w in waits[:-1]:
                        nop = mybir.InstNoOp(
                            name=f"I-dmawaitfix-{ctr}", ins=[], outs=[]
                        )
                        ctr += 1
                        nop.engine = inst.engine
                        nop.sync_info = mybir.SyncInfo(on_wait=[w], on_update=[])
                        out.append(nop)
                    inst.sync_info = mybir.SyncInfo(
                        on_wait=[waits[-1]], on_update=list(si.on_update)
                    )
                    changed = True
                out.append(inst)
            if changed:
                blk.instructions = out
    return ctr


def _build(nrep=1):
    nc = bacc.Bacc("TRN2", target_bir_lowering=False, num_devices=N_CORES,
                   dynamic_dma_scratch_size=2048)

    # ---- I/O (host pre-layouts everything partition-major) ----
    qr = nc.dram_tensor("qr", [128, NDC, S], BF16, kind="ExternalInput")
    kr = nc.dram_tensor("kr", [128, NDC, S], BF16, kind="ExternalInput")
    vr = nc.dram_tensor("vr", [128, NDC, S], BF16, kind="ExternalInput")
    wq = nc.dram_tensor("wq", [128, NDC, FLOC], BF16, kind="ExternalInput")
    wk = nc.dram_tensor("wk", [128, NDC, DK], BF16, kind="ExternalInput")
    wv = nc.dram_tensor("wv", [128, NDC, DK], BF16, kind="ExternalInput")
    wd = nc.dram_tensor("wd", [128, NDC, D], BF16, kind="ExternalInput")
    masks = nc.dram_tensor("masks", [128, 4, SB], BF16, kind="ExternalInput")
    ident = nc.dram_tensor("ident", [128, 128], BF16, kind="ExternalInput")
    outS = nc.dram_tensor("outS", [SLOC, D], F32, kind="ExternalOutput")

    with TileContext(nc) as tc:
        with (
            tc.tile_pool(name="consts", bufs=1) as consts,
            tc.tile_pool(name="wqp", bufs=2) as wqp,
            tc.tile_pool(name="wkvp", bufs=2) as wkvp,
            tc.tile_pool(name="streamA", bufs=12) as streamA,
            tc.tile_pool(name="ktp", bufs=5) as ktp,
            tc.tile_pool(name="vtp", bufs=5) as vtp,
            tc.tile_pool(name="bigacc", bufs=1) as bigacc,
            tc.tile_pool(name="finals", bufs=1) as finals,
            tc.tile_pool(name="epool", bufs=3) as epool,
            tc.tile_pool(name="eaccp", bufs=2) as eaccp,
            tc.tile_pool(name="recp", bufs=2) as recp,
            tc.tile_pool(name="atout", bufs=2) as atout,
            tc.tile_pool(name="featp", bufs=2) as featp,
            tc.tile_pool(name="psA", bufs=2, space="PSUM") as psA,
            tc.tile_pool(name="psB", bufs=2, space="PSUM") as psB,
            tc.tile_pool(name="dram", bufs=1, space="DRAM") as dram,
        ):
            def one_rep(rep):
                ones_sb = consts.tile([128, 128], BF16, name="ones_sb")
                nc.vector.memset(ones_sb[:], 1.0)
                # PE warmup: dummy matmuls ramp the PE p-state while the
                # first stream DMAs are in flight
                warm = psB.tile([128, 2, SB], F32, name="warm", tag="psB")
                for w in range(24):
                    nc.tensor.matmul(warm[:, w % 2, 0:128], lhsT=ones_sb[:],
                                     rhs=ones_sb[:], start=(w < 2),
                                     stop=(w >= 22))

                # persistent activations
                QT_sb = finals.tile([128, NH_LOC, S], BF16, name="QT_sb")
                KT_sb = finals.tile([128, S], BF16, name="KT_sb")
                VT_sb = finals.tile([128, S], BF16, name="VT_sb")
                V_sb = finals.tile([128, NST, DK], BF16, name="V_sb")

                # a2a bounce buffers (one per local head)
                ain = [dram.tile([N_CORES, 128, SLOC], BF16,
                                 name=f"ain{h}", tag=f"ain{h}")
                       for h in range(NH_LOC)]
                aout = [dram.tile([N_CORES, 128, SLOC], BF16,
                                  name=f"aout{h}", tag=f"aout{h}")
                        for h in range(NH_LOC)]

                def drain(dst_flat, na, srcs, mode):
                    # dst_flat: AP [128, na*SB*len(srcs)]; srcs: psum tiles
                    # [128, na, SB]; mode: "copy" | "add" | None->into dst
                    for t, src in enumerate(srcs):
                        dst = dst_flat[:, t * na * SB:(t + 1) * na * SB] \
                            .rearrange("p (a b) -> p a b", a=na)
                        if mode == "copy":
                            nc.vector.tensor_copy(dst, src[:])
                        else:
                            nc.vector.tensor_tensor(dst, dst, src[:],
                                                    mybir.AluOpType.add)

                # ---- P1: interleaved projections ----
                for blk in range(NBLK):
                    dc0 = blk * BLK
                    # K weights + K stream first (first consumers), then V, Q
                    wkc = wkvp.tile([128, BLK, DK], BF16, name="wkc", tag="wkc")
                    nc.sync.dma_start(wkc[:], wk[:, dc0:dc0 + BLK, :])
                    kt = []
                    vt = []
                    qt = []
                    for i in range(BLK):
                        t = ktp.tile([128, S], BF16, name="kt_c", tag="kt")
                        nc.sync.dma_start(t[:], kr[:, dc0 + i, :])
                        kt.append(t)
                    wvc = wkvp.tile([128, BLK, DK], BF16, name="wvc", tag="wvc")
                    nc.sync.dma_start(wvc[:], wv[:, dc0:dc0 + BLK, :])
                    wqc = wqp.tile([128, BLK, FLOC], BF16, name="wqc", tag="wqc")
                    nc.sync.dma_start(wqc[:], wq[:, dc0:dc0 + BLK, :])
                    for i in range(BLK):
                        t = vtp.tile([128, S], BF16, name="vt_c", tag="vt")
                        nc.sync.dma_start(t[:], vr[:, dc0 + i, :])
                        vt.append(t)
                    for i in range(BLK):
                        t = streamA.tile([128, S], BF16, name="qt_c", tag="sa")
                        nc.sync.dma_start(t[:], qr[:, dc0 + i, :])
                        qt.append(t)

                    # K -> psA pair
                    kps = [psA.tile([128, 2, SB], F32, name=f"kps{t}", tag="psA")
                           for t in range(2)]
                    for i in range(BLK):
                        for sb in range(NSB):
                            nc.tensor.matmul(
                                kps[sb // 2][:, sb % 2, :],
                                lhsT=wkc[:, i, :],
                                rhs=kt[i][:, sb * SB:(sb + 1) * SB],
                                start=(i == 0), stop=(i == BLK - 1),
                            )
                    drain(KT_sb[:], 2, kps, "copy" if blk == 0 else "add")

                    # V -> psB pair, bf16 accumulate directly in VT_sb
                    vps = [psB.tile([128, 2, SB], F32, name=f"vps{t}", tag="psB")
                           for t in range(2)]
                    for i in range(BLK):
                        for sb in range(NSB):
                            nc.tensor.matmul(
                                vps[sb // 2][:, sb % 2, :],
                                lhsT=wvc[:, i, :],
                                rhs=vt[i][:, sb * SB:(sb + 1) * SB],
                                start=(i == 0), stop=(i == BLK - 1),
                            )
                    drain(VT_sb[:], 2, vps, "copy" if blk == 0 else "add")

                    # Q heads: h0,h2 -> psA pair; h1,h3 -> psB pair
                    for f in range(NH_LOC):
                        pool = psA if f % 2 == 0 else psB
                        tag = "psA" if f % 2 == 0 else "psB"
                        qps = [pool.tile([128, 2, SB], F32, name=f"qps{t}",
                                         tag=tag) for t in range(2)]
                        for i in range(BLK):
                            for sb in range(NSB):
                                nc.tensor.matmul(
                                    qps[sb // 2][:, sb % 2, :],
                                    lhsT=wqc[:, i, f * 128:(f + 1) * 128],
                                    rhs=qt[i][:, sb * SB:(sb + 1) * SB],
                                    start=(i == 0), stop=(i == BLK - 1),
                                )
                        drain(QT_sb[:, f, :], 2, qps,
                              "copy" if blk == 0 else "add")

                # V transposes: VT_sb [dv, s] -> V_sb [s, kt, dv]
                ident_sb = consts.tile([128, 128], BF16, name="ident_sb")
                nc.sync.dma_start(ident_sb[:], ident[:])
                masks_sb = consts.tile([128, 4, SB], BF16, name="masks_sb")
                nc.sync.dma_start(masks_sb[:], masks[:])
                for t in range(2):
                    tp = psB.tile([128, 2, SB], BF16, name="tp", tag="psB")
                    for i in range(8):
                        st = t * 8 + i
                        nc.tensor.transpose(
                            tp[:, i // 4, (i % 4) * 128:(i % 4 + 1) * 128],
                            VT_sb[:, st * 128:(st + 1) * 128], ident_sb[:])
                    nc.vector.tensor_copy(
                        V_sb[:, t * 8:(t + 1) * 8, :].rearrange(
                            "p (a b) c -> p a (b c)", a=2),
                        tp[:])

                # prefetch pass-0 wd pair-chunks while attention runs (SP
                # queue is past all P1 stream DMAs at this point)
                wdpre = {}
                for oqp in range(2):
                    for j in range(N_CORES):
                        if len(wdpre) >= 12:
                            break
                        t = streamA.tile([128, 2, OQ], BF16, name="wdq",
                                         tag="sa")
                        nc.sync.dma_start(
                            t[:], wd[:, 0 * 8 + j,
                                     2 * oqp * OQ:(2 * oqp + 2) * OQ]
                            .rearrange("p (a b) -> p a b", a=2))
                        wdpre[(0, oqp, j)] = t

                # ---- P2: attention ----
                # Two heads interleaved with a one-qb STAGGER: while one head
                # is at its shallow qb boundary (den/rec/normalize tail), the
                # other is mid-qb with deep PE work, so boundary latency never
                # idles the PE. Denominators via DVE accumulation.
                def attn_qb(h, qb):
                    nkt = 4 * qb + 4
                    ngrp = nkt // 2
                    pvden = psB.tile([128, 2, SB], F32, name=f"pvden{h}",
                                     tag="psB")
                    eacc = eaccp.tile([128, 2, SB], BF16, name=f"eacc{h}",
                                      tag=f"eacc{h % 2}")
                    order = list(range(ngrp))
                    if ngrp > 2:  # diagonal (masked) groups first
                        order = [ngrp - 2, ngrp - 1] + list(range(ngrp - 2))
                    for pos, g in enumerate(order):
                        first, last = pos == 0, pos == ngrp - 1
                        # second diagonal group: cols < SLOC fully masked;
                        # skip them in exp/mask/PV/eacc entirely
                        rq = SLOC if (g == ngrp - 1 and ngrp > 2) else 0
                        sc = psA.tile([128, 2, SB], F32, name="sc", tag="psA")
                        for i in range(2):
                            kt_i = 2 * g + i
                            nc.tensor.matmul(
                                sc[:, i, :],
                                lhsT=KT_sb[:, kt_i * 128:(kt_i + 1) * 128],
                                rhs=QT_sb[:, h, qb * SB:(qb + 1) * SB],
                                start=True, stop=True,
                            )
                        E = epool.tile([128, 2, SB], BF16, name="E", tag="E")
                        nc.scalar.activation(
                            E[:, :, rq:SB], sc[:, :, rq:SB],
                            mybir.ActivationFunctionType.Exp)
                        if g >= ngrp - 2:  # diagonal pair -> causal mask
                            u = g - (ngrp - 2)
                            nc.vector.tensor_tensor(
                                E[:, :, rq:SB], E[:, :, rq:SB],
                                masks_sb[:, 2 * u:2 * u + 2, rq:SB],
                                mybir.AluOpType.mult)
                        for i in range(2):
                            nc.tensor.matmul(
                                pvden[:, 0, rq:SB],
                                lhsT=V_sb[:, 2 * g + i, :],
                                rhs=E[:, i, rq:SB],
                                start=(first and i == 0),
                                stop=(last and i == 1),
                            )
                        if first:
                            nc.vector.tensor_copy(eacc[:], E[:])
                        else:
                            nc.vector.tensor_tensor(
                                eacc[:, :, rq:SB], eacc[:, :, rq:SB],
                                E[:, :, rq:SB], mybir.AluOpType.add)
                        yield
                    # qb tail: denominator matmuls, normalize, a2a input
                    nc.tensor.matmul(pvden[:, 1, :], lhsT=ones_sb[:],
                                     rhs=eacc[:, 0, :], start=True, stop=False)
                    nc.tensor.matmul(pvden[:, 1, :], lhsT=ones_sb[:],
                                     rhs=eacc[:, 1, :], start=False, stop=True)
                    rec = recp.tile([128, SB], F32, name="rec", tag="rec")
                    nc.vector.reciprocal(rec[:], pvden[:, 1, :])
                    attn_t = atout.tile([128, SB], BF16, name="attn_t",
                                        tag="attn")
                    nc.vector.tensor_tensor(attn_t[:], pvden[:, 0, :],
                                            rec[:], mybir.AluOpType.mult)
                    nc.sync.dma_start(ain[h][2 * qb], attn_t[:, 0:SLOC])
                    nc.sync.dma_start(ain[h][2 * qb + 1], attn_t[:, SLOC:SB])
                    yield

                for hp in range(NH_LOC // 2):
                    ha, hb = 2 * hp, 2 * hp + 1
                    def fire_a2a(h):
                        nc.gpsimd.collective_compute(
                            "AllToAll",
                            mybir.AluOpType.bypass,
                            replica_groups=[list(range(N_CORES))],
                            ins=[ain[h][:]],
                            outs=[aout[h][:]],
                        )

                    for s in range(NSB + 1):
                        active = []
                        if s < NSB:
                            active.append(attn_qb(ha, s))
                        if s >= 1:
                            active.append(attn_qb(hb, s - 1))
                        while active:
                            nxt = []
                            for gen in active:
                                try:
                                    next(gen)
                                    nxt.append(gen)
                                except StopIteration:
                                    pass
                            active = nxt
                        if s == NSB - 1:
                            fire_a2a(ha)  # head a is done one step early
                    fire_a2a(hb)

                # ---- P3: output projection, one pass per head-group ----
                out_acc = bigacc.tile([128, 2, D], F32, name="out_acc",
                                      tag="big")
                for h in range(NH_LOC):
                    feats = featp.tile([128, N_CORES, SLOC], BF16,
                                       name="feats", tag="feats")
                    for j in range(N_CORES):
                        nc.sync.dma_start(feats[:, j, :], aout[h][j])
                    for oq in range(4):
                        oqp = oq // 2
                        if oq % 2 == 0:
                            wdqp = []
                            for j in range(N_CORES):
                                if (h, oqp, j) in wdpre:
                                    wdqp.append(wdpre[(h, oqp, j)])
                                    continue
                                t = streamA.tile([128, 2, OQ], BF16,
                                                 name="wdq", tag="sa")
                                nc.sync.dma_start(
                                    t[:], wd[:, h * 8 + j,
                                             2 * oqp * OQ:(2 * oqp + 2) * OQ]
                                    .rearrange("p (a b) -> p a b", a=2))
                                wdqp.append(t)
                        wdq = [wt[:, oq % 2, :] for wt in wdqp]
                        pool = psA if oq % 2 == 0 else psB
                        tag = "psA" if oq % 2 == 0 else "psB"
                        pq = [pool.tile([128, 2, SB], F32, name=f"ops{t}",
                                        tag=tag) for t in range(2)]
                        for qh in range(2):
                            for j in range(N_CORES):
                                for t in range(2):
                                    nc.tensor.matmul(
                                        pq[qh][:, t, :],
                                        lhsT=feats[:, j,
                                                   qh * 128:(qh + 1) * 128],
                                        rhs=wdq[j][:, t * SB:(t + 1) * SB],
                                        start=(j == 0),
                                        stop=(j == N_CORES - 1),
                                    )
                        for qh in range(2):
                            dst = out_acc[:, qh, oq * OQ:(oq + 1) * OQ] \
                                .rearrange("p (b c) -> p b c", c=SB)
                            if h == 0:
                                nc.vector.tensor_copy(dst, pq[qh][:])
                            else:
                                nc.vector.tensor_tensor(dst, dst, pq[qh][:],
                                                        mybir.AluOpType.add)
                            if h == NH_LOC - 1:
                                nc.sync.dma_start(
                                    outS[qh * 128:(qh + 1) * 128,
                                         oq * OQ:(oq + 1) * OQ],
                                    out_acc[:, qh, oq * OQ:(oq + 1) * OQ])

            for rep in range(nrep):
                one_rep(rep)

    nc.compile()
    _legalize_dma_waits(nc)
    nc.codegen_inst_isa_subclasses()
    return nc


_NC_CACHE = None


def _get_nc():
    global _NC_CACHE
    if _NC_CACHE is None:
        _NC_CACHE = _build()
    return _NC_CACHE


def _pm(a, nchunk, width):
    """[nchunk*128, width] -> [128, nchunk, width] partition-major bf16."""
    bf = ml_dtypes.bfloat16
    return np.ascontiguousarray(
        a.reshape(nchunk, 128, width).transpose(1, 0, 2)).astype(bf)


def _make_in_maps(q, k, v, Wq, Wk, Wv, Wd):
    bf = ml_dtypes.bfloat16
    scale = np.float32(DK) ** -0.5
    qT = np.ascontiguousarray(q.reshape(S, D).T)   # [D, S]
    kT = np.ascontiguousarray(k.reshape(S, D).T)
    vT = np.ascontiguousarray(v.reshape(S, D).T)
    qr = _pm(qT, NDC, S)
    kr = _pm(kT, NDC, S)
    vr = _pm(vT, NDC, S)

    # permuted Wd: row block (h, j) = features of global head 4j+h
    wdT = np.ascontiguousarray(Wd.T)               # [feats, od]
    blocks = []
    for h in range(NH_LOC):
        for j in range(N_CORES):
            g = 4 * j + h
            blocks.append(wdT[g * 128:(g + 1) * 128, :])
    wd_r = _pm(np.concatenate(blocks, axis=0), NDC, D)

    kp = np.arange(128, dtype=np.int32)[:, None]
    qf = np.arange(SB, dtype=np.int32)[None, :]
    masks = np.stack(
        [(qf >= kp + 128 * d).astype(np.float32) for d in range(4)], axis=1
    ).astype(bf)  # [128, 4, SB]
    ident = np.eye(128, dtype=np.float32).astype(bf)

    in_maps = []
    for c in range(N_CORES):
        fs = slice(FLOC * c, FLOC * (c + 1))
        ks = slice(DK * c, DK * (c + 1))
        in_maps.append({
            "qr": qr,
            "kr": kr,
            "vr": vr,
            "wq": _pm(np.ascontiguousarray((Wq[fs, :] * scale).T), NDC, FLOC),
            "wk": _pm(np.ascontiguousarray(Wk[ks, :].T), NDC, DK),
            "wv": _pm(np.ascontiguousarray(Wv[ks, :].T), NDC, DK),
            "wd": wd_r,
            "masks": masks,
            "ident": ident,
        })
    return in_maps


def _assemble(results):
    return np.concatenate(
        [r["outS"] for r in results], axis=0).reshape(1, S, D)


def kernel(q, k, v, Wq, Wk, Wv, Wd, _trace=False, **_ignored):
    nc = _get_nc()
    in_maps = _make_in_maps(
        np.asarray(q, np.float32), np.asarray(k, np.float32),
        np.asarray(v, np.float32), np.asarray(Wq, np.float32),
        np.asarray(Wk, np.float32), np.asarray(Wv, np.float32),
        np.asarray(Wd, np.float32),
    )
    res = run_bass_kernel_spmd(
        nc, in_maps, core_ids=list(range(N_CORES)), trace=_trace
    )
    out = _assemble(res.results)
    if _trace:
        return out, res
    return out


# revision 56
# speedup vs baseline: 40470.2536x; 1.0242x over previous
"""Trainium2 Bass kernel for GQA MultiHeadAttention (B=1, S=2048, D=4096,
H=32 query heads, HKV=8 kv heads, DK=DV=128) on 8 NeuronCores.

Sharding: core c owns query heads 4c..4c+3 and kv head c for the projections
and attention (tensor-parallel over heads); the output projection is
sequence-sharded: a per-head AllToAll redistributes the attention output so
core c holds all 4096 attention features for its 256 sequence columns, then
each core computes out[:, own 256 cols] against the full (permuted) Wd.

Phase layout per core:
  P1 interleaved projections: per 8-chunk block of the D contraction,
     K, V, Qh0..3 round-robin over two 2-bank PSUM pools, partial sums
     accumulated in SBUF; kT/vT/qT chunks stream interleaved so the PE
     never waits on any single tensor's DMA.
  P2 attention: head pairs interleaved with a one-qb stagger so one
     head's softmax tail always overlaps the other's matmul-dense middle;
     scores in double-buffered 2-k-tile groups -> one batched exp each;
     causal mask on DVE (second diagonal group width-restricted); PV
     accumulated in PSUM across the row; softmax denominator accumulated
     on DVE + one ones-matmul. Per-head AllToAll fires as soon as that
     head finishes.
  P3 output projection, one pass per head-group, od-quarters so wd
     sub-chunks stay resident across both q-halves; accumulates into an
     SBUF fp32 buffer reusing the Q-accumulator slot; full Wd streamed
     through the same pool as the qT stream.

Self-contained: hardcodes all shapes; inputs are the full unsharded tensors
keyed as in the problem's setup_inputs().
"""

import numpy as np
import ml_dtypes

import concourse.bacc as bacc
import concourse.mybir as mybir
from concourse.tile import TileContext
from concourse.bass_utils import run_bass_kernel_spmd

BF16 = mybir.dt.bfloat16
F32 = mybir.dt.float32

N_CORES = 8
S = 2048            # sequence length
D = 4096            # model dim
DK = 128            # head dim
NH_LOC = 4          # query heads per core
FLOC = NH_LOC * DK  # per-core attention features (512)
NDC = D // 128      # contraction chunks of 128 over D (32)
SB = 512            # q/s block width
NSB = S // SB       # 4
NST = S // 128      # 16 seq tiles of 128
SLOC = S // N_CORES  # per-core output seq columns (256)
NBLK = 4            # projection blocks
BLK = NDC // NBLK   # 8 dc per block
OQ = 1024           # output-projection od quarter width

_DMA_TYPES = ("InstDMACopy", "InstDMATranspose")


def _legalize_dma_waits(nc):
    """DMA pseudo-instructions encode at most ONE sem wait (the ISA events
    slot). If Tile's sem assignment leaves more on a DMA, walrus rejects it
    ("Too many sync wait commands"). Hoist all but the last wait onto fresh
    nop instructions inserted immediately before the DMA on the same engine —
    the sequencer executes them in order, so semantics are identical."""
    ctr = 0
    for f in nc.m.functions:
        for blk in f.blocks:
            out = []
            changed = False
            for inst in blk.instructions:
                si = inst.sync_info
                if (
                    si is not None
                    and len(si.on_wait) > 1
                    and type(inst).__name__ in _DMA_TYPES
                ):
                    waits = list(si.on_wait)
                    for w in waits[:-1]:
                        nop = mybir.InstNoOp(
                            name=f"I-dmawaitfix-{ctr}", ins=[], outs=[]
                        )
                        ctr += 1
                        nop.engine = inst.engine
                        nop.sync_info = mybir.SyncInfo(on_wait=[w], on_update=[])
                        out.append(nop)
                    inst.sync_info = mybir.SyncInfo(
                        on_wait=[waits[-1]], on_update=list(si.on_update)
                    )
                    changed = True
                out.append(inst)
            if changed:
                blk.instructions = out
    return ctr


def _build(nrep=1):
    nc = bacc.Bacc("TRN2", target_bir_lowering=False, num_devices=N_CORES,
                   dynamic_dma_scratch_size=2048)

    # ---- I/O (host pre-layouts everything partition-major) ----
    qr = nc.dram_tensor("qr", [128, NDC, S], BF16, kind="ExternalInput")
    kr = nc.dram_tensor("kr", [128, NDC, S], BF16, kind="ExternalInput")
    vr = nc.dram_tensor("vr", [128, NDC, S], BF16, kind="ExternalInput")
    wq = nc.dram_tensor("wq", [128, NDC, FLOC], BF16, kind="ExternalInput")
    wk = nc.dram_tensor("wk", [128, NDC, DK], BF16, kind="ExternalInput")
    wv = nc.dram_tensor("wv", [128, NDC, DK], BF16, kind="ExternalInput")
    wd = nc.dram_tensor("wd", [128, NDC, D], BF16, kind="ExternalInput")
    masks = nc.dram_tensor("masks", [128, 4, SB], BF16, kind="ExternalInput")
    ident = nc.dram_tensor("ident", [128, 128], BF16, kind="ExternalInput")
    outS = nc.dram_tensor("outS", [SLOC, D], F32, kind="ExternalOutput")

    with TileContext(nc) as tc:
        with (
            tc.tile_pool(name="consts", bufs=1) as consts,
            tc.tile_pool(name="wqp", bufs=2) as wqp,
            tc.tile_pool(name="wkvp", bufs=2) as wkvp,
            tc.tile_pool(name="streamA", bufs=16) as streamA,
            tc.tile_pool(name="ktp", bufs=5) as ktp,
            tc.tile_pool(name="vtp", bufs=5) as vtp,
            tc.tile_pool(name="bigacc", bufs=1) as bigacc,
            tc.tile_pool(name="finals", bufs=1) as finals,
            tc.tile_pool(name="epool", bufs=3) as epool,
            tc.tile_pool(name="eaccp", bufs=2) as eaccp,
            tc.tile_pool(name="recp", bufs=2) as recp,
            tc.tile_pool(name="atout", bufs=2) as atout,
            tc.tile_pool(name="featp", bufs=2) as featp,
            tc.tile_pool(name="psA", bufs=2, space="PSUM") as psA,
            tc.tile_pool(name="psB", bufs=2, space="PSUM") as psB,
            tc.tile_pool(name="dram", bufs=1, space="DRAM") as dram,
        ):
            def one_rep(rep):
                ones_sb = consts.tile([128, 128], BF16, name="ones_sb")
                nc.vector.memset(ones_sb[:], 1.0)
                # PE warmup: dummy matmuls ramp the PE p-state while the
                # first stream DMAs are in flight
                warm = psB.tile([128, 2, SB], F32, name="warm", tag="psB")
                for w in range(24):
                    nc.tensor.matmul(warm[:, w % 2, 0:128], lhsT=ones_sb[:],
                                     rhs=ones_sb[:], start=(w < 2),
                                     stop=(w >= 22))

                # persistent activations
                QT_sb = finals.tile([128, NH_LOC, S], BF16, name="QT_sb")
                KT_sb = finals.tile([128, S], BF16, name="KT_sb")
                VT_tile = featp.tile([128, N_CORES, SLOC], BF16,
                                     name="VT_sb", tag="feats")
                VT_sb = VT_tile[:].rearrange("p a b -> p (a b)")
                V_sb = finals.tile([128, NST, DK], BF16, name="V_sb")

                # a2a bounce buffers (one per local head)
                ain = [dram.tile([N_CORES, 128, SLOC], BF16,
                                 name=f"ain{h}", tag=f"ain{h}")
                       for h in range(NH_LOC)]
                aout = [dram.tile([N_CORES, 128, SLOC], BF16,
                                  name=f"aout{h}", tag=f"aout{h}")
                        for h in range(NH_LOC)]

                def drain(dst_flat, na, srcs, mode, eng=None):
                    # dst_flat: AP [128, na*SB*len(srcs)]; srcs: psum tiles
                    # [128, na, SB]; mode: "copy" | "add"
                    eng = eng or nc.vector
                    for t, src in enumerate(srcs):
                        dst = dst_flat[:, t * na * SB:(t + 1) * na * SB] \
                            .rearrange("p (a b) -> p a b", a=na)
                        if mode == "copy":
                            eng.tensor_copy(dst, src[:])
                        else:
                            eng.tensor_tensor(dst, dst, src[:],
                                              mybir.AluOpType.add)

                # ---- P1: interleaved projections ----
                for blk in range(NBLK):
                    dc0 = blk * BLK
                    # K weights + K stream first (first consumers), then V, Q
                    wkc = wkvp.tile([128, BLK, DK], BF16, name="wkc", tag="wkc")
                    nc.sync.dma_start(wkc[:], wk[:, dc0:dc0 + BLK, :])
                    kt = []
                    vt = []
                    qt = []
                    for i in range(BLK):
                        t = ktp.tile([128, S], BF16, name="kt_c", tag="kt")
                        nc.sync.dma_start(t[:], kr[:, dc0 + i, :])
                        kt.append(t)
                    wvc = wkvp.tile([128, BLK, DK], BF16, name="wvc", tag="wvc")
                    nc.sync.dma_start(wvc[:], wv[:, dc0:dc0 + BLK, :])
                    wqc = wqp.tile([128, BLK, FLOC], BF16, name="wqc", tag="wqc")
                    nc.sync.dma_start(wqc[:], wq[:, dc0:dc0 + BLK, :])
                    for i in range(BLK):
                        t = vtp.tile([128, S], BF16, name="vt_c", tag="vt")
                        nc.sync.dma_start(t[:], vr[:, dc0 + i, :])
                        vt.append(t)
                    for i in range(BLK):
                        t = streamA.tile([128, S], BF16, name="qt_c", tag="sa")
                        nc.sync.dma_start(t[:], qr[:, dc0 + i, :])
                        qt.append(t)

                    # K -> psA pair
                    kps = [psA.tile([128, 2, SB], F32, name=f"kps{t}", tag="psA")
                           for t in range(2)]
                    for i in range(BLK):
                        for sb in range(NSB):
                            nc.tensor.matmul(
                                kps[sb // 2][:, sb % 2, :],
                                lhsT=wkc[:, i, :],
                                rhs=kt[i][:, sb * SB:(sb + 1) * SB],
                                start=(i == 0), stop=(i == BLK - 1),
                            )
                    drain(KT_sb[:], 2, kps, "copy" if blk == 0 else "add")

                    # V -> psB pair, bf16 accumulate directly in VT_sb
                    vps = [psB.tile([128, 2, SB], F32, name=f"vps{t}", tag="psB")
                           for t in range(2)]
                    for i in range(BLK):
                        for sb in range(NSB):
                            nc.tensor.matmul(
                                vps[sb // 2][:, sb % 2, :],
                                lhsT=wvc[:, i, :],
                                rhs=vt[i][:, sb * SB:(sb + 1) * SB],
                                start=(i == 0), stop=(i == BLK - 1),
                            )
                    drain(VT_sb, 2, vps, "copy" if blk == 0 else "add")

                    # Q heads: h0,h2 -> psA pair; h1,h3 -> psB pair
                    for f in range(NH_LOC):
                        pool = psA if f % 2 == 0 else psB
                        tag = "psA" if f % 2 == 0 else "psB"
                        qps = [pool.tile([128, 2, SB], F32, name=f"qps{t}",
                                         tag=tag) for t in range(2)]
                        for i in range(BLK):
                            for sb in range(NSB):
                                nc.tensor.matmul(
                                    qps[sb // 2][:, sb % 2, :],
                                    lhsT=wqc[:, i, f * 128:(f + 1) * 128],
                                    rhs=qt[i][:, sb * SB:(sb + 1) * SB],
                                    start=(i == 0), stop=(i == BLK - 1),
                                )
                        drain(QT_sb[:, f, :], 2, qps,
                              "copy" if blk == 0 else "add")

                # V transposes: VT_sb [dv, s] -> V_sb [s, kt, dv]
                ident_sb = consts.tile([128, 128], BF16, name="ident_sb")
                nc.sync.dma_start(ident_sb[:], ident[:])
                masks_sb = consts.tile([128, 4, SB], BF16, name="masks_sb")
                nc.sync.dma_start(masks_sb[:], masks[:])
                for t in range(2):
                    tp = psB.tile([128, 2, SB], BF16, name="tp", tag="psB")
                    for i in range(8):
                        st = t * 8 + i
                        nc.tensor.transpose(
                            tp[:, i // 4, (i % 4) * 128:(i % 4 + 1) * 128],
                            VT_sb[:, st * 128:(st + 1) * 128], ident_sb[:])
                    nc.vector.tensor_copy(
                        V_sb[:, t * 8:(t + 1) * 8, :].rearrange(
                            "p (a b) c -> p a (b c)", a=2),
                        tp[:])

                # prefetch pass-0 wd pair-chunks while attention runs (SP
                # queue is past all P1 stream DMAs at this point)
                wdpre = {}
                for oqp in range(2):
                    for j in range(N_CORES):
                        if len(wdpre) >= 12:
                            break
                        t = streamA.tile([128, 2, OQ], BF16, name="wdq",
                                         tag="sa")
                        nc.sync.dma_start(
                            t[:], wd[:, 0 * 8 + j,
                                     2 * oqp * OQ:(2 * oqp + 2) * OQ]
                            .rearrange("p (a b) -> p a b", a=2))
                        wdpre[(0, oqp, j)] = t

                # ---- P2: attention ----
                # Two heads interleaved with a one-qb STAGGER: while one head
                # is at its shallow qb boundary (den/rec/normalize tail), the
                # other is mid-qb with deep PE work, so boundary latency never
                # idles the PE. Denominators via DVE accumulation.
                def attn_qb(h, qb):
                    nkt = 4 * qb + 4
                    ngrp = nkt // 2
                    pvden = psB.tile([128, 2, SB], F32, name=f"pvden{h}",
                                     tag="psB")
                    eacc = eaccp.tile([128, 2, SB], BF16, name=f"eacc{h}",
                                      tag=f"eacc{h % 2}")
                    order = list(range(ngrp))
                    if ngrp > 2:  # diagonal (masked) groups first
                        order = [ngrp - 2, ngrp - 1] + list(range(ngrp - 2))
                    for pos, g in enumerate(order):
                        first, last = pos == 0, pos == ngrp - 1
                        # second diagonal group: cols < SLOC fully masked;
                        # skip them in exp/mask/PV/eacc entirely
                        rq = SLOC if (g == ngrp - 1 and ngrp > 2) else 0
                        sc = psA.tile([128, 2, SB], F32, name="sc", tag="psA")
                        for i in range(2):
                            kt_i = 2 * g + i
                            nc.tensor.matmul(
                                sc[:, i, :],
                                lhsT=KT_sb[:, kt_i * 128:(kt_i + 1) * 128],
                                rhs=QT_sb[:, h, qb * SB:(qb + 1) * SB],
                                start=True, stop=True,
                            )
                        E = epool.tile([128, 2, SB], BF16, name="E", tag="E")
                        nc.scalar.activation(
                            E[:, :, rq:SB], sc[:, :, rq:SB],
                            mybir.ActivationFunctionType.Exp)
                        if g >= ngrp - 2:  # diagonal pair -> causal mask
                            u = g - (ngrp - 2)
                            nc.vector.tensor_tensor(
                                E[:, :, rq:SB], E[:, :, rq:SB],
                                masks_sb[:, 2 * u:2 * u + 2, rq:SB],
                                mybir.AluOpType.mult)
                        for i in range(2):
                            nc.tensor.matmul(
                                pvden[:, 0, rq:SB],
                                lhsT=V_sb[:, 2 * g + i, :],
                                rhs=E[:, i, rq:SB],
                                start=(first and i == 0),
                                stop=(last and i == 1),
                            )
                        if first:
                            nc.vector.tensor_copy(eacc[:], E[:])
                        else:
                            nc.vector.tensor_tensor(
                                eacc[:, :, rq:SB], eacc[:, :, rq:SB],
                                E[:, :, rq:SB], mybir.AluOpType.add)
                        yield
                    # qb tail: denominator matmuls, normalize, a2a input
                    nc.tensor.matmul(pvden[:, 1, :], lhsT=ones_sb[:],
                                     rhs=eacc[:, 0, :], start=True, stop=False)
                    nc.tensor.matmul(pvden[:, 1, :], lhsT=ones_sb[:],
                                     rhs=eacc[:, 1, :], start=False, stop=True)
                    rec = recp.tile([128, SB], F32, name="rec", tag="rec")
                    nc.vector.reciprocal(rec[:], pvden[:, 1, :])
                    attn_t = atout.tile([128, SB], BF16, name="attn_t",
                                        tag="attn")
                    nc.vector.tensor_tensor(attn_t[:], pvden[:, 0, :],
                                            rec[:], mybir.AluOpType.mult)
                    nc.sync.dma_start(ain[h][2 * qb], attn_t[:, 0:SLOC])
                    nc.sync.dma_start(ain[h][2 * qb + 1], attn_t[:, SLOC:SB])
                    yield

                def fire_a2a(h):
                    nc.gpsimd.collective_compute(
                        "AllToAll",
                        mybir.AluOpType.bypass,
                        replica_groups=[list(range(N_CORES))],
                        ins=[ain[h][:]],
                        outs=[aout[h][:]],
                    )

                for hp in range(NH_LOC // 2):
                    ha, hb = 2 * hp, 2 * hp + 1
                    for s in range(NSB + 1):
                        active = []
                        if s < NSB:
                            active.append(attn_qb(ha, s))
                        if s >= 1:
                            active.append(attn_qb(hb, s - 1))
                        while active:
                            nxt = []
                            for gen in active:
                                try:
                                    next(gen)
                                    nxt.append(gen)
                                except StopIteration:
                                    pass
                            active = nxt
                        if s == NSB - 1:
                            fire_a2a(ha)  # head a is done one step early
                    fire_a2a(hb)

                # ---- P3: output projection, one pass per head-group ----
                out_acc = bigacc.tile([128, 2, D], F32, name="out_acc",
                                      tag="big")
                for h in range(NH_LOC):
                    feats = featp.tile([128, N_CORES, SLOC], BF16,
                                       name="feats", tag="feats")
                    for j in range(N_CORES):
                        nc.sync.dma_start(feats[:, j, :], aout[h][j])
                    for oq in range(4):
                        oqp = oq // 2
                        if oq % 2 == 0:
                            wdqp = []
                            for j in range(N_CORES):
                                if (h, oqp, j) in wdpre:
                                    wdqp.append(wdpre[(h, oqp, j)])
                                    continue
                                t = streamA.tile([128, 2, OQ], BF16,
                                                 name="wdq", tag="sa")
                                nc.sync.dma_start(
                                    t[:], wd[:, h * 8 + j,
                                             2 * oqp * OQ:(2 * oqp + 2) * OQ]
                                    .rearrange("p (a b) -> p a b", a=2))
                                wdqp.append(t)
                        wdq = [wt[:, oq % 2, :] for wt in wdqp]
                        pool = psA if oq % 2 == 0 else psB
                        tag = "psA" if oq % 2 == 0 else "psB"
                        pq = [pool.tile([128, 2, SB], F32, name=f"ops{t}",
                                        tag=tag) for t in range(2)]
                        for qh in range(2):
                            for j in range(N_CORES):
                                for t in range(2):
                                    nc.tensor.matmul(
                                        pq[qh][:, t, :],
                                        lhsT=feats[:, j,
                                                   qh * 128:(qh + 1) * 128],
                                        rhs=wdq[j][:, t * SB:(t + 1) * SB],
                                        start=(j == 0),
                                        stop=(j == N_CORES - 1),
                                    )
                        for qh in range(2):
                            dst = out_acc[:, qh, oq * OQ:(oq + 1) * OQ] \
                                .rearrange("p (b c) -> p b c", c=SB)
                            if h == 0:
                                nc.vector.tensor_copy(dst, pq[qh][:])
                            else:
                                nc.vector.tensor_tensor(dst, dst, pq[qh][:],
                                                        mybir.AluOpType.add)
                            if h == NH_LOC - 1:
                                nc.sync.dma_start(
                                    outS[qh * 128:(qh + 1) * 128,
                                         oq * OQ:(oq + 1) * OQ],
                                    out_acc[:, qh, oq * OQ:(oq + 1) * OQ])

            for rep in range(nrep):
                one_rep(rep)

    nc.compile()
    _legalize_dma_waits(nc)
    nc.codegen_inst_isa_subclasses()
    return nc


_NC_CACHE = None


def _get_nc():
    global _NC_CACHE
    if _NC_CACHE is None:
        _NC_CACHE = _build()
    return _NC_CACHE


def _pm(a, nchunk, width):
    """[nchunk*128, width] -> [128, nchunk, width] partition-major bf16."""
    bf = ml_dtypes.bfloat16
    return np.ascontiguousarray(
        a.reshape(nchunk, 128, width).transpose(1, 0, 2)).astype(bf)


def _make_in_maps(q, k, v, Wq, Wk, Wv, Wd):
    bf = ml_dtypes.bfloat16
    scale = np.float32(DK) ** -0.5
    qT = np.ascontiguousarray(q.reshape(S, D).T)   # [D, S]
    kT = np.ascontiguousarray(k.reshape(S, D).T)
    vT = np.ascontiguousarray(v.reshape(S, D).T)
    qr = _pm(qT, NDC, S)
    kr = _pm(kT, NDC, S)
    vr = _pm(vT, NDC, S)

    # permuted Wd: row block (h, j) = features of global head 4j+h
    wdT = np.ascontiguousarray(Wd.T)               # [feats, od]
    blocks = []
    for h in range(NH_LOC):
        for j in range(N_CORES):
            g = 4 * j + h
            blocks.append(wdT[g * 128:(g + 1) * 128, :])
    wd_r = _pm(np.concatenate(blocks, axis=0), NDC, D)

    kp = np.arange(128, dtype=np.int32)[:, None]
    qf = np.arange(SB, dtype=np.int32)[None, :]
    masks = np.stack(
        [(qf >= kp + 128 * d).astype(np.float32) for d in range(4)], axis=1
    ).astype(bf)  # [128, 4, SB]
    ident = np.eye(128, dtype=np.float32).astype(bf)

    in_maps = []
    for c in range(N_CORES):
        fs = slice(FLOC * c, FLOC * (c + 1))
        ks = slice(DK * c, DK * (c + 1))
        in_maps.append({
            "qr": qr,
            "kr": kr,
            "vr": vr,
            "wq": _pm(np.ascontiguousarray((Wq[fs, :] * scale).T), NDC, FLOC),
            "wk": _pm(np.ascontiguousarray(Wk[ks, :].T), NDC, DK),
            "wv": _pm(np.ascontiguousarray(Wv[ks, :].T), NDC, DK),
            "wd": wd_r,
            "masks": masks,
            "ident": ident,
        })
    return in_maps


def _assemble(results):
    return np.concatenate(
        [r["outS"] for r in results], axis=0).reshape(1, S, D)


def kernel(q, k, v, Wq, Wk, Wv, Wd, _trace=False, **_ignored):
    nc = _get_nc()
    in_maps = _make_in_maps(
        np.asarray(q, np.float32), np.asarray(k, np.float32),
        np.asarray(v, np.float32), np.asarray(Wq, np.float32),
        np.asarray(Wk, np.float32), np.asarray(Wv, np.float32),
        np.asarray(Wd, np.float32),
    )
    res = run_bass_kernel_spmd(
        nc, in_maps, core_ids=list(range(N_CORES)), trace=_trace
    )
    out = _assemble(res.results)
    if _trace:
        return out, res
    return out


# revision 63
# speedup vs baseline: 40526.3910x; 1.0014x over previous
"""Trainium2 Bass kernel for GQA MultiHeadAttention (B=1, S=2048, D=4096,
H=32 query heads, HKV=8 kv heads, DK=DV=128) on 8 NeuronCores.

Sharding: core c owns query heads 4c..4c+3 and kv head c for the projections
and attention (tensor-parallel over heads); the output projection is
sequence-sharded: a per-head AllToAll redistributes the attention output so
core c holds all 4096 attention features for its 256 sequence columns, then
each core computes out[:, own 256 cols] against the full (permuted) Wd.

Phase layout per core:
  P1 interleaved projections: per 8-chunk block of the D contraction,
     K, V, Qh0..3 round-robin over two 2-bank PSUM pools, partial sums
     accumulated in SBUF; kT/vT/qT chunks stream interleaved so the PE
     never waits on any single tensor's DMA.
  P2 attention: head pairs interleaved with a one-qb stagger so one
     head's softmax tail always overlaps the other's matmul-dense middle;
     scores in double-buffered 2-k-tile groups -> one batched exp each;
     causal mask on DVE (second diagonal group width-restricted); PV
     accumulated in PSUM across the row; softmax denominator accumulated
     on DVE + one ones-matmul. Per-head AllToAll fires as soon as that
     head finishes.
  P3 output projection, one pass per head-group, od-quarters so wd
     sub-chunks stay resident across both q-halves; accumulates into an
     SBUF fp32 buffer reusing the Q-accumulator slot; full Wd streamed
     through the same pool as the qT stream.

Self-contained: hardcodes all shapes; inputs are the full unsharded tensors
keyed as in the problem's setup_inputs().
"""

import numpy as np
import ml_dtypes

import concourse.bacc as bacc
import concourse.mybir as mybir
from concourse.tile import TileContext
from concourse.bass_utils import run_bass_kernel_spmd

BF16 = mybir.dt.bfloat16
F32 = mybir.dt.float32

N_CORES = 8
S = 2048            # sequence length
D = 4096            # model dim
DK = 128            # head dim
NH_LOC = 4          # query heads per core
FLOC = NH_LOC * DK  # per-core attention features (512)
NDC = D // 128      # contraction chunks of 128 over D (32)
SB = 512            # q/s block width
NSB = S // SB       # 4
NST = S // 128      # 16 seq tiles of 128
SLOC = S // N_CORES  # per-core output seq columns (256)
NBLK = 4            # projection blocks
BLK = NDC // NBLK   # 8 dc per block
OQ = 1024           # output-projection od quarter width

_DMA_TYPES = ("InstDMACopy", "InstDMATranspose")


def _legalize_dma_waits(nc):
    """DMA pseudo-instructions encode at most ONE sem wait (the ISA events
    slot). If Tile's sem assignment leaves more on a DMA, walrus rejects it
    ("Too many sync wait commands"). Hoist all but the last wait onto fresh
    nop instructions inserted immediately before the DMA on the same engine —
    the sequencer executes them in order, so semantics are identical."""
    ctr = 0
    for f in nc.m.functions:
        for blk in f.blocks:
            out = []
            changed = False
            for inst in blk.instructions:
                si = inst.sync_info
                if (
                    si is not None
                    and len(si.on_wait) > 1
                    and type(inst).__name__ in _DMA_TYPES
                ):
                    waits = list(si.on_wait)
                    for w in waits[:-1]:
                        nop = mybir.InstNoOp(
                            name=f"I-dmawaitfix-{ctr}", ins=[], outs=[]
                        )
                        ctr += 1
                        nop.engine = inst.engine
                        nop.sync_info = mybir.SyncInfo(on_wait=[w], on_update=[])
                        out.append(nop)
                    inst.sync_info = mybir.SyncInfo(
                        on_wait=[waits[-1]], on_update=list(si.on_update)
                    )
                    changed = True
                out.append(inst)
            if changed:
                blk.instructions = out
    return ctr


def _build(nrep=1):
    nc = bacc.Bacc("TRN2", target_bir_lowering=False, num_devices=N_CORES,
                   dynamic_dma_scratch_size=2048)

    # ---- I/O (host pre-layouts everything partition-major) ----
    qr = nc.dram_tensor("qr", [128, NDC, S], BF16, kind="ExternalInput")
    kr = nc.dram_tensor("kr", [128, NDC, S], BF16, kind="ExternalInput")
    vr = nc.dram_tensor("vr", [128, NDC, S], BF16, kind="ExternalInput")
    wq = nc.dram_tensor("wq", [128, NDC, FLOC], BF16, kind="ExternalInput")
    wk = nc.dram_tensor("wk", [128, NDC, DK], BF16, kind="ExternalInput")
    wv = nc.dram_tensor("wv", [128, NDC, DK], BF16, kind="ExternalInput")
    wd = nc.dram_tensor("wd", [128, NDC, D], BF16, kind="ExternalInput")
    masks = nc.dram_tensor("masks", [128, 4, SB], BF16, kind="ExternalInput")
    ident = nc.dram_tensor("ident", [128, 128], BF16, kind="ExternalInput")
    outS = nc.dram_tensor("outS", [SLOC, D], F32, kind="ExternalOutput")

    with TileContext(nc) as tc:
        with (
            tc.tile_pool(name="consts", bufs=1) as consts,
            tc.tile_pool(name="wqp", bufs=2) as wqp,
            tc.tile_pool(name="wkvp", bufs=2) as wkvp,
            tc.tile_pool(name="streamA", bufs=16) as streamA,
            tc.tile_pool(name="ktp", bufs=5) as ktp,
            tc.tile_pool(name="vtp", bufs=5) as vtp,
            tc.tile_pool(name="bigacc", bufs=1) as bigacc,
            tc.tile_pool(name="finals", bufs=1) as finals,
            tc.tile_pool(name="epool", bufs=3) as epool,
            tc.tile_pool(name="eaccp", bufs=2) as eaccp,
            tc.tile_pool(name="recp", bufs=2) as recp,
            tc.tile_pool(name="atout", bufs=2) as atout,
            tc.tile_pool(name="featp", bufs=2) as featp,
            tc.tile_pool(name="psA", bufs=2, space="PSUM") as psA,
            tc.tile_pool(name="psB", bufs=2, space="PSUM") as psB,
            tc.tile_pool(name="dram", bufs=1, space="DRAM") as dram,
        ):
            def one_rep(rep):
                ones_sb = consts.tile([128, 128], BF16, name="ones_sb")
                nc.vector.memset(ones_sb[:], 1.0)
                # PE warmup: dummy matmuls ramp the PE p-state while the
                # first stream DMAs are in flight
                warm = psB.tile([128, 2, SB], F32, name="warm", tag="psB")
                for w in range(24):
                    nc.tensor.matmul(warm[:, w % 2, 0:128], lhsT=ones_sb[:],
                                     rhs=ones_sb[:], start=(w < 2),
                                     stop=(w >= 22))

                # persistent activations
                QT_sb = finals.tile([128, NH_LOC, S], BF16, name="QT_sb")
                KT_sb = finals.tile([128, S], BF16, name="KT_sb")
                VT_tile = featp.tile([128, N_CORES, SLOC], BF16,
                                     name="VT_sb", tag="feats")
                VT_sb = VT_tile[:].rearrange("p a b -> p (a b)")
                V_sb = finals.tile([128, NST, DK], BF16, name="V_sb")

                # a2a bounce buffers (one per local head)
                ain = [dram.tile([N_CORES, 128, SLOC], BF16,
                                 name=f"ain{h}", tag=f"ain{h}")
                       for h in range(NH_LOC)]
                aout = [dram.tile([N_CORES, 128, SLOC], BF16,
                                  name=f"aout{h}", tag=f"aout{h}")
                        for h in range(NH_LOC)]

                def drain(dst_flat, na, srcs, mode, eng=None):
                    # dst_flat: AP [128, na*SB*len(srcs)]; srcs: psum tiles
                    # [128, na, SB]; mode: "copy" | "add"
                    eng = eng or nc.vector
                    for t, src in enumerate(srcs):
                        dst = dst_flat[:, t * na * SB:(t + 1) * na * SB] \
                            .rearrange("p (a b) -> p a b", a=na)
                        if mode == "copy":
                            eng.tensor_copy(dst, src[:])
                        else:
                            eng.tensor_tensor(dst, dst, src[:],
                                              mybir.AluOpType.add)

                # ---- P1: interleaved projections ----
                for blk in range(NBLK):
                    dc0 = blk * BLK
                    # K weights + K stream first (first consumers), then V, Q
                    wkc = wkvp.tile([128, BLK, DK], BF16, name="wkc", tag="wkc")
                    nc.sync.dma_start(wkc[:], wk[:, dc0:dc0 + BLK, :])
                    kt = []
                    vt = []
                    qt = []
                    for i in range(BLK):
                        t = ktp.tile([128, S], BF16, name="kt_c", tag="kt")
                        nc.sync.dma_start(t[:], kr[:, dc0 + i, :])
                        kt.append(t)
                    wvc = wkvp.tile([128, BLK, DK], BF16, name="wvc", tag="wvc")
                    nc.sync.dma_start(wvc[:], wv[:, dc0:dc0 + BLK, :])
                    wqc = wqp.tile([128, BLK, FLOC], BF16, name="wqc", tag="wqc")
                    nc.sync.dma_start(wqc[:], wq[:, dc0:dc0 + BLK, :])
                    for i in range(BLK):
                        t = vtp.tile([128, S], BF16, name="vt_c", tag="vt")
                        nc.sync.dma_start(t[:], vr[:, dc0 + i, :])
                        vt.append(t)
                    for i in range(BLK):
                        t = streamA.tile([128, S], BF16, name="qt_c", tag="sa")
                        nc.sync.dma_start(t[:], qr[:, dc0 + i, :])
                        qt.append(t)

                    # K -> psA pair
                    kps = [psA.tile([128, 2, SB], F32, name=f"kps{t}", tag="psA")
                           for t in range(2)]
                    for i in range(BLK):
                        for sb in range(NSB):
                            nc.tensor.matmul(
                                kps[sb // 2][:, sb % 2, :],
                                lhsT=wkc[:, i, :],
                                rhs=kt[i][:, sb * SB:(sb + 1) * SB],
                                start=(i == 0), stop=(i == BLK - 1),
                            )
                    drain(KT_sb[:], 2, kps, "copy" if blk == 0 else "add")

                    # V -> psB pair, bf16 accumulate directly in VT_sb
                    vps = [psB.tile([128, 2, SB], F32, name=f"vps{t}", tag="psB")
                           for t in range(2)]
                    for i in range(BLK):
                        for sb in range(NSB):
                            nc.tensor.matmul(
                                vps[sb // 2][:, sb % 2, :],
                                lhsT=wvc[:, i, :],
                                rhs=vt[i][:, sb * SB:(sb + 1) * SB],
                                start=(i == 0), stop=(i == BLK - 1),
                            )
                    drain(VT_sb, 2, vps, "copy" if blk == 0 else "add")

                    # Q heads: h0,h2 -> psA pair; h1,h3 -> psB pair
                    for f in range(NH_LOC):
                        pool = psA if f % 2 == 0 else psB
                        tag = "psA" if f % 2 == 0 else "psB"
                        qps = [pool.tile([128, 2, SB], F32, name=f"qps{t}",
                                         tag=tag) for t in range(2)]
                        for i in range(BLK):
                            for sb in range(NSB):
                                nc.tensor.matmul(
                                    qps[sb // 2][:, sb % 2, :],
                                    lhsT=wqc[:, i, f * 128:(f + 1) * 128],
                                    rhs=qt[i][:, sb * SB:(sb + 1) * SB],
                                    start=(i == 0), stop=(i == BLK - 1),
                                )
                        drain(QT_sb[:, f, :], 2, qps,
                              "copy" if blk == 0 else "add")

                # V transposes: VT_sb [dv, s] -> V_sb [s, kt, dv]
                ident_sb = consts.tile([128, 128], BF16, name="ident_sb")
                nc.sync.dma_start(ident_sb[:], ident[:])
                masks_sb = consts.tile([128, 4, SB], BF16, name="masks_sb")
                nc.sync.dma_start(masks_sb[:], masks[:])
                for t in range(2):
                    tp = psB.tile([128, 2, SB], BF16, name="tp", tag="psB")
                    for i in range(8):
                        st = t * 8 + i
                        nc.tensor.transpose(
                            tp[:, i // 4, (i % 4) * 128:(i % 4 + 1) * 128],
                            VT_sb[:, st * 128:(st + 1) * 128], ident_sb[:])
                    nc.vector.tensor_copy(
                        V_sb[:, t * 8:(t + 1) * 8, :].rearrange(
                            "p (a b) c -> p a (b c)", a=2),
                        tp[:])

                # prefetch pass-0 wd pair-chunks while attention runs (SP
                # queue is past all P1 stream DMAs at this point)
                wdpre = {}
                for oqp in range(2):
                    for j in range(N_CORES):
                        if len(wdpre) >= 12:
                            break
                        t = streamA.tile([128, 2, OQ], BF16, name="wdq",
                                         tag="sa")
                        nc.sync.dma_start(
                            t[:], wd[:, 0 * 8 + j,
                                     2 * oqp * OQ:(2 * oqp + 2) * OQ]
                            .rearrange("p (a b) -> p a b", a=2))
                        wdpre[(0, oqp, j)] = t

                # ---- P2: attention ----
                # Two heads interleaved with a one-qb STAGGER: while one head
                # is at its shallow qb boundary (den/rec/normalize tail), the
                # other is mid-qb with deep PE work, so boundary latency never
                # idles the PE. Denominators via DVE accumulation.
                def attn_qb(h, qb):
                    nkt = 4 * qb + 4
                    ngrp = nkt // 2
                    pvden = psB.tile([128, 2, SB], F32, name=f"pvden{h}",
                                     tag="psB")
                    eacc = eaccp.tile([128, 2, SB], BF16, name=f"eacc{h}",
                                      tag=f"eacc{h % 2}")
                    order = list(range(ngrp))
                    if ngrp > 2:  # diagonal (masked) groups first
                        order = [ngrp - 2, ngrp - 1] + list(range(ngrp - 2))
                    for pos, g in enumerate(order):
                        first, last = pos == 0, pos == ngrp - 1
                        # second diagonal group: cols < SLOC fully masked;
                        # skip them in exp/mask/PV/eacc entirely
                        rq = SLOC if (g == ngrp - 1 and ngrp > 2) else 0
                        sc = psA.tile([128, 2, SB], F32, name="sc", tag="psA")
                        for i in range(2):
                            kt_i = 2 * g + i
                            nc.tensor.matmul(
                                sc[:, i, :],
                                lhsT=KT_sb[:, kt_i * 128:(kt_i + 1) * 128],
                                rhs=QT_sb[:, h, qb * SB:(qb + 1) * SB],
                                start=True, stop=True,
                            )
                        E = epool.tile([128, 2, SB], BF16, name="E", tag="E")
                        nc.scalar.activation(
                            E[:, :, rq:SB], sc[:, :, rq:SB],
                            mybir.ActivationFunctionType.Exp)
                        if g >= ngrp - 2:  # diagonal pair -> causal mask
                            u = g - (ngrp - 2)
                            nc.vector.tensor_tensor(
                                E[:, :, rq:SB], E[:, :, rq:SB],
                                masks_sb[:, 2 * u:2 * u + 2, rq:SB],
                                mybir.AluOpType.mult)
                        for i in range(2):
                            nc.tensor.matmul(
                                pvden[:, 0, rq:SB],
                                lhsT=V_sb[:, 2 * g + i, :],
                                rhs=E[:, i, rq:SB],
                                start=(first and i == 0),
                                stop=(last and i == 1),
                            )
                        if first:
                            nc.vector.tensor_copy(eacc[:], E[:])
                        else:
                            nc.vector.tensor_tensor(
                                eacc[:, :, rq:SB], eacc[:, :, rq:SB],
                                E[:, :, rq:SB], mybir.AluOpType.add)
                        yield
                    # qb tail: denominator matmuls, normalize, a2a input
                    nc.tensor.matmul(pvden[:, 1, :], lhsT=ones_sb[:],
                                     rhs=eacc[:, 0, :], start=True, stop=False)
                    nc.tensor.matmul(pvden[:, 1, :], lhsT=ones_sb[:],
                                     rhs=eacc[:, 1, :], start=False, stop=True)
                    rec = recp.tile([128, SB], F32, name="rec", tag="rec")
                    nc.vector.reciprocal(rec[:], pvden[:, 1, :])
                    attn_t = atout.tile([128, SB], BF16, name="attn_t",
                                        tag="attn")
                    nc.vector.tensor_tensor(attn_t[:], pvden[:, 0, :],
                                            rec[:], mybir.AluOpType.mult)
                    nc.sync.dma_start(ain[h][2 * qb], attn_t[:, 0:SLOC])
                    nc.sync.dma_start(ain[h][2 * qb + 1], attn_t[:, SLOC:SB])
                    yield

                def fire_a2a(h):
                    nc.gpsimd.collective_compute(
                        "AllToAll",
                        mybir.AluOpType.bypass,
                        replica_groups=[list(range(N_CORES))],
                        ins=[ain[h][:]],
                        outs=[aout[h][:]],
                    )

                for hp in range(NH_LOC // 2):
                    ha, hb = 2 * hp, 2 * hp + 1
                    for s in range(NSB + 1):
                        active = []
                        if s < NSB:
                            active.append(attn_qb(ha, s))
                        if s >= 1:
                            active.append(attn_qb(hb, s - 1))
                        while active:
                            nxt = []
                            for gen in active:
                                try:
                                    next(gen)
                                    nxt.append(gen)
                                except StopIteration:
                                    pass
                            active = nxt
                        if s == NSB - 1:
                            fire_a2a(ha)  # head a is done one step early
                    fire_a2a(hb)

                # ---- P3: output projection, one pass per head-group ----
                out_acc = bigacc.tile([128, 2, D], F32, name="out_acc",
                                      tag="big")
                for h in range(NH_LOC):
                    feats = featp.tile([128, N_CORES, SLOC], BF16,
                                       name="feats", tag="feats")
                    for j in range(N_CORES):
                        nc.sync.dma_start(feats[:, j, :], aout[h][j])
                    for oq in range(4):
                        oqp = oq // 2
                        if oq % 2 == 0:
                            wdqp = []
                            for j in range(N_CORES):
                                if (h, oqp, j) in wdpre:
                                    wdqp.append(wdpre[(h, oqp, j)])
                                    continue
                                t = streamA.tile([128, 2, OQ], BF16,
                                                 name="wdq", tag="sa")
                                nc.sync.dma_start(
                                    t[:], wd[:, h * 8 + j,
                                             2 * oqp * OQ:(2 * oqp + 2) * OQ]
                                    .rearrange("p (a b) -> p a b", a=2))
                                wdqp.append(t)
                        wdq = [wt[:, oq % 2, :] for wt in wdqp]
                        pool = psA if oq % 2 == 0 else psB
                        tag = "psA" if oq % 2 == 0 else "psB"
                        pq = [pool.tile([128, 2, SB], F32, name=f"ops{t}",
                                        tag=tag) for t in range(2)]
                        for qh in range(2):
                            for j in range(N_CORES):
                                for t in range(2):
                                    nc.tensor.matmul(
                                        pq[qh][:, t, :],
                                        lhsT=feats[:, j,
                                                   qh * 128:(qh + 1) * 128],
                                        rhs=wdq[j][:, t * SB:(t + 1) * SB],
                                        start=(j == 0),
                                        stop=(j == N_CORES - 1),
                                    )
                        for qh in range(2):
                            if h == NH_LOC - 1 and oq == 3:
                                # very last quarter: drain+store in halves to
                                # shorten the kernel tail
                                for t in range(2):
                                    o0 = oq * OQ + t * SB
                                    dsth = out_acc[:, qh, o0:o0 + SB]
                                    nc.vector.tensor_tensor(
                                        dsth, dsth, pq[qh][:, t, :],
                                        mybir.AluOpType.add)
                                    nc.sync.dma_start(
                                        outS[qh * 128:(qh + 1) * 128,
                                             o0:o0 + SB], dsth)
                                continue
                            dst = out_acc[:, qh, oq * OQ:(oq + 1) * OQ] \
                                .rearrange("p (b c) -> p b c", c=SB)
                            if h == 0:
                                nc.vector.tensor_copy(dst, pq[qh][:])
                            else:
                                nc.vector.tensor_tensor(dst, dst, pq[qh][:],
                                                        mybir.AluOpType.add)
                            if h == NH_LOC - 1:
                                nc.sync.dma_start(
                                    outS[qh * 128:(qh + 1) * 128,
                                         oq * OQ:(oq + 1) * OQ],
                                    out_acc[:, qh, oq * OQ:(oq + 1) * OQ])

            for rep in range(nrep):
                one_rep(rep)

    nc.compile()
    _legalize_dma_waits(nc)
    nc.codegen_inst_isa_subclasses()
    return nc


_NC_CACHE = None


def _get_nc():
    global _NC_CACHE
    if _NC_CACHE is None:
        _NC_CACHE = _build()
    return _NC_CACHE


def _pm(a, nchunk, width):
    """[nchunk*128, width] -> [128, nchunk, width] partition-major bf16."""
    bf = ml_dtypes.bfloat16
    return np.ascontiguousarray(
        a.reshape(nchunk, 128, width).transpose(1, 0, 2)).astype(bf)


def _make_in_maps(q, k, v, Wq, Wk, Wv, Wd):
    bf = ml_dtypes.bfloat16
    scale = np.float32(DK) ** -0.5
    qT = np.ascontiguousarray(q.reshape(S, D).T)   # [D, S]
    kT = np.ascontiguousarray(k.reshape(S, D).T)
    vT = np.ascontiguousarray(v.reshape(S, D).T)
    qr = _pm(qT, NDC, S)
    kr = _pm(kT, NDC, S)
    vr = _pm(vT, NDC, S)

    # permuted Wd: row block (h, j) = features of global head 4j+h
    wdT = np.ascontiguousarray(Wd.T)               # [feats, od]
    blocks = []
    for h in range(NH_LOC):
        for j in range(N_CORES):
            g = 4 * j + h
            blocks.append(wdT[g * 128:(g + 1) * 128, :])
    wd_r = _pm(np.concatenate(blocks, axis=0), NDC, D)

    kp = np.arange(128, dtype=np.int32)[:, None]
    qf = np.arange(SB, dtype=np.int32)[None, :]
    masks = np.stack(
        [(qf >= kp + 128 * d).astype(np.float32) for d in range(4)], axis=1
    ).astype(bf)  # [128, 4, SB]
    ident = np.eye(128, dtype=np.float32).astype(bf)

    in_maps = []
    for c in range(N_CORES):
        fs = slice(FLOC * c, FLOC * (c + 1))
        ks = slice(DK * c, DK * (c + 1))
        in_maps.append({
            "qr": qr,
            "kr": kr,
            "vr": vr,
            "wq": _pm(np.ascontiguousarray((Wq[fs, :] * scale).T), NDC, FLOC),
            "wk": _pm(np.ascontiguousarray(Wk[ks, :].T), NDC, DK),
            "wv": _pm(np.ascontiguousarray(Wv[ks, :].T), NDC, DK),
            "wd": wd_r,
            "masks": masks,
            "ident": ident,
        })
    return in_maps


def _assemble(results):
    return np.concatenate(
        [r["outS"] for r in results], axis=0).reshape(1, S, D)


def kernel(q, k, v, Wq, Wk, Wv, Wd, _trace=False, **_ignored):
    nc = _get_nc()
    in_maps = _make_in_maps(
        np.asarray(q, np.float32), np.asarray(k, np.float32),
        np.asarray(v, np.float32), np.asarray(Wq, np.float32),
        np.asarray(Wk, np.float32), np.asarray(Wv, np.float32),
        np.asarray(Wd, np.float32),
    )
    res = run_bass_kernel_spmd(
        nc, in_maps, core_ids=list(range(N_CORES)), trace=_trace
    )
    out = _assemble(res.results)
    if _trace:
        return out, res
    return out
